# revision 11
# baseline (speedup 1.0000x reference)
"""Trainium2 Bass kernel for a dense transformer block (pre-LN, causal MHA + FFN).

Reference shapes: x [B=2, T=2048, D=1024], H=16 heads, HD=64, FF=4096, f32.

Sharding (8 NeuronCores, SPMD single program):
  - Tokens (B*T = 32 blocks of 128) are owned by cores: core c owns blocks
    (c, 15-c) of each batch -> 512 tokens/core. LN1/LN2, residuals, proj and
    FFN are pure data-parallel over tokens.
  - Attention is head-sharded (2 heads/core over ALL tokens) so the causal
    loop structure is identical on every core. Two cheap collectives glue the
    shardings: AllGather of the LN1 output (feature-major, bf16, 1MB/rank)
    before QKV, and AllToAll of the attention output (1MB/rank) after.
  - Weights are cast to bf16 on host; LN gains/biases are folded into the
    adjacent weight matrices on host. Matmuls run in bf16 (fp32 accumulate),
    the residual stream stays f32 on device.
"""
import numpy as np
import ml_dtypes

import concourse.bass as bass
from concourse import bacc
import concourse.tile as tile
import concourse.mybir as mybir
from concourse.bass_utils import run_bass_kernel_spmd

F32 = mybir.dt.float32
BF16 = mybir.dt.bfloat16
AF = mybir.ActivationFunctionType
OP = mybir.AluOpType

P = 128
NCORES = 8

STAGES = ["ln1", "ag", "qkv", "attn", "a2a", "proj", "ln2", "ff1", "ff2"]


def _owner_map(b_total, nb):
    """(batch, block) -> (core, local block idx). Full size pairs block i with
    nb-1-i on one core so causal attention work is balanced."""
    own = {}
    if nb == 2 * NCORES:
        for b in range(b_total):
            for qb in range(nb):
                c = qb if qb < NCORES else nb - 1 - qb
                own[(b, qb)] = (c, 2 * b + (0 if qb < NCORES else 1))
    else:
        for b in range(b_total):
            for qb in range(nb):
                inst = b * nb + qb
                own[(b, qb)] = (inst % NCORES, inst // NCORES)
    return own


def build(T=2048, B=2, D=1024, H=16, FF=4096, eps=1e-5, stop_after=None, attn_sub=5):
    HD = D // H
    NB = T // P                    # blocks per batch
    HPC = H // NCORES              # heads per core
    NBLK = B * NB // NCORES        # owned token blocks per core
    TOK = NBLK * P                 # owned tokens per core
    TQ = NCORES * TOK              # all tokens (B*T)
    DK = D // P                    # d_model chunks
    NFF = FF // P                  # ff chunks
    NKB = B * NB                   # total key blocks
    own = _owner_map(B, NB)
    GW = HPC * HD                  # my attention feature width (128)
    assert GW == P

    if stop_after is None:
        stop_after = "ff2"
    assert stop_after in STAGES

    def active(stage):
        return STAGES.index(stage) <= STAGES.index(stop_after)

    def blkidx(b, qb):
        return own[(b, qb)]

    def tokcol(b, qb):
        c, i = blkidx(b, qb)
        return c * TOK + i * P

    nc = bacc.Bacc("TRN2", num_devices=NCORES)

    # ---- DRAM I/O (per core) ----
    xin = nc.dram_tensor("xin", [NBLK, P, D], F32, kind="ExternalInput").ap()
    wqkv = nc.dram_tensor("wqkv", [DK, P, 3 * GW], BF16, kind="ExternalInput").ap()
    qkvb = nc.dram_tensor("qkvb", [3 * GW], F32, kind="ExternalInput").ap()
    wproj = nc.dram_tensor("wproj", [DK, P, D], BF16, kind="ExternalInput").ap()
    bproj = nc.dram_tensor("bproj", [D], F32, kind="ExternalInput").ap()
    w1 = nc.dram_tensor("w1", [DK, P, FF], BF16, kind="ExternalInput").ap()
    b1e = nc.dram_tensor("b1e", [FF], F32, kind="ExternalInput").ap()
    w2 = nc.dram_tensor("w2", [NFF, P, D], BF16, kind="ExternalInput").ap()
    b2 = nc.dram_tensor("b2", [D], F32, kind="ExternalInput").ap()
    out = nc.dram_tensor("out", [NBLK, P, D], F32, kind="ExternalOutput").ap()

    def bcast_rows(vec_ap):
        return bass.AP(tensor=vec_ap.tensor, offset=vec_ap.offset,
                       ap=[[0, P]] + list(vec_ap.ap))

    with tile.TileContext(nc) as tc:
        with (
            tc.tile_pool(name="persist", bufs=1) as persist,
            tc.tile_pool(name="dram", bufs=1, space="DRAM") as dram,
            tc.tile_pool(name="wstream", bufs=3) as wstream,
            tc.tile_pool(name="work", bufs=3) as work,
            tc.tile_pool(name="small", bufs=4) as small,
        ):
            # ---------------- stage A: LN1 over own tokens ----------------
            x_res = persist.tile([P, NBLK, D], F32)
            h_tm = persist.tile([P, NBLK, D], BF16, tag="tm_buf")
            eps_t = persist.tile([P, 1], F32)
            nc.vector.memset(eps_t[:], eps)
            for t in range(NBLK):
                nc.sync.dma_start(x_res[:, t, :], xin[t])
                st = small.tile([P, 2, 6], F32, tag="ln_st")
                xg = x_res[:, t, :].rearrange("p (s f) -> p s f", s=2)
                for s in range(2):
                    nc.vector.bn_stats(out=st[:, s, :], in_=xg[:, s, :])
                mv = small.tile([P, 2], F32, tag="ln_mv")
                nc.vector.bn_aggr(out=mv[:], in_=st[:])
                rstd = small.tile([P, 1], F32, tag="ln_rstd")
                nc.scalar.activation(rstd[:], mv[:, 1:2], AF.Sqrt, bias=eps_t[:])
                nc.vector.reciprocal(rstd[:], rstd[:])
                nc.vector.tensor_scalar(
                    h_tm[:, t, :], x_res[:, t, :],
                    scalar1=mv[:, 0:1], scalar2=rstd[:],
                    op0=OP.subtract, op1=OP.mult)

            # transpose h -> feature-major [P, DK, TOK] via DMA transpose
            hT_own = persist.tile([P, DK, TOK], BF16, tag="fm_buf")
            for t in range(NBLK):
                for d in range(DK):
                    nc.sync.dma_start(
                        hT_own[:, d, t * P:(t + 1) * P],
                        h_tm[:, t, d * P:(d + 1) * P],
                        transpose=True)

            if active("ag"):
                # ---------------- AllGather hT ----------------
                ag_in = dram.tile([DK, P, TOK], BF16)
                ag_out = dram.tile([NCORES, DK, P, TOK], BF16)
                for d in range(DK):
                    nc.sync.dma_start(ag_in[d], hT_own[:, d, :])
                nc.gpsimd.collective_compute(
                    "AllGather", OP.bypass,
                    replica_groups=[list(range(NCORES))],
                    ins=[ag_in.opt()], outs=[ag_out.opt()])

                hT = persist.tile([P, DK, TQ], BF16, tag="big_buf")
                for r in range(NCORES):
                    for d in range(DK):
                        nc.sync.dma_start(
                            hT[:, d, r * TOK:(r + 1) * TOK], ag_out[r, d])

            if active("qkv"):
                # ---------------- stage B: QKV for my heads ----------------
                qT = persist.tile([P, TQ], BF16)
                kT = persist.tile([P, TQ], BF16)
                qkvb_sb = persist.tile([P, 3], F32)
                nc.sync.dma_start(qkvb_sb[:],
                                  qkvb.rearrange("(g p) -> p g", p=P))
                wqkv_sb = persist.tile([P, DK, 3 * GW], BF16)
                nc.sync.dma_start(wqkv_sb[:], wqkv.rearrange("d p f -> p d f"))
                NT = TQ // 512
                with tc.tile_pool(name="ps1", bufs=2, space="PSUM") as ps1:
                    for g, dst in ((0, qT), (1, kT)):
                        for nt in range(NT):
                            ps = ps1.tile([P, 512], F32, tag="mm", name="ps")
                            for d in range(DK):
                                nc.tensor.matmul(
                                    ps[:], wqkv_sb[:, d, g * P:(g + 1) * P],
                                    hT[:, d, nt * 512:(nt + 1) * 512],
                                    start=(d == 0), stop=(d == DK - 1))
                            nc.vector.tensor_scalar_add(
                                dst[:, nt * 512:(nt + 1) * 512], ps[:],
                                scalar1=qkvb_sb[:, g:g + 1])

                    # V token-major with fused ones column
                    # (folded LN1 bias for V must be zero; host asserts)
                    v_aug = persist.tile([P, NKB * HPC, HD + 1], BF16)
                    nc.vector.memset(v_aug[:, :, HD:HD + 1], 1.0)
                    for b in range(B):
                        for kb in range(NB):
                            col = tokcol(b, kb)
                            slot = (b * NB + kb) * HPC
                            ps = ps1.tile([P, 512], F32, tag="mm", name="ps")
                            for d in range(DK):
                                nc.tensor.matmul(
                                    ps[:, 0:GW], hT[:, d, col:col + P],
                                    wqkv_sb[:, d, 2 * P:3 * P],
                                    start=(d == 0), stop=(d == DK - 1))
                            for sub in range(HPC):
                                nc.vector.tensor_copy(
                                    v_aug[:, slot + sub, 0:HD],
                                    ps[:, sub * HD:(sub + 1) * HD])

            if active("attn"):
                # ------------- stage C: attention (my HPC heads) -----------
                tril = persist.tile([P, P], BF16)
                nc.vector.memset(tril[:], 1.0)
                nc.gpsimd.affine_select(
                    out=tril[:], in_=tril[:], compare_op=OP.is_ge,
                    fill=0.0, base=0, channel_multiplier=-1, pattern=[[1, P]])

                a2a_in = dram.tile([NCORES, NBLK, P, GW], BF16)
                a2a_out = dram.tile([NCORES, NBLK, P, GW], BF16)

                with (
                    tc.tile_pool(name="ps_s", bufs=2, space="PSUM") as ps_sp,
                    tc.tile_pool(name="ps_av", bufs=2, space="PSUM") as ps_avp,
                ):
                    for b in range(B):
                        for qb in range(NB):
                            dst_c, dst_i = blkidx(b, qb)
                            qcol = tokcol(b, qb)
                            pav = [ps_avp.tile([P, 512], F32, tag=f"av{s}",
                                               name=f"pav{s}")[:, 0:HD + 1]
                                   for s in range(HPC)]
                            for kb in range(qb + 1):
                                kcol = tokcol(b, kb)
                                ps_s = [ps_sp.tile([P, 512], F32,
                                                   tag=f"score{s}",
                                                   name=f"ps_s{s}")[:, 0:P]
                                        for s in range(HPC)]
                                if attn_sub < 2:
                                    continue
                                for sub in range(HPC):
                                    nc.tensor.matmul(
                                        ps_s[sub][:],
                                        kT[sub * HD:(sub + 1) * HD,
                                           kcol:kcol + P],
                                        qT[sub * HD:(sub + 1) * HD,
                                           qcol:qcol + P],
                                        start=True, stop=True)
                                p_sb = work.tile([P, HPC * P], BF16,
                                                 tag="p_sb", name="p_sb")
                                for sub in range(HPC):
                                    nc.scalar.activation(
                                        p_sb[:, sub * P:(sub + 1) * P],
                                        ps_s[sub][:], AF.Exp,
                                        scale=float(HD) ** -0.5)
                                if kb == qb and attn_sub >= 3:
                                    for sub in range(HPC):
                                        nc.vector.tensor_tensor(
                                            p_sb[:, sub * P:(sub + 1) * P],
                                            p_sb[:, sub * P:(sub + 1) * P],
                                            tril[:], OP.mult)
                                slot = (b * NB + kb) * HPC
                                if attn_sub >= 4:
                                    for sub in range(HPC):
                                        nc.tensor.matmul(
                                            pav[sub][:],
                                            p_sb[:, sub * P:(sub + 1) * P],
                                            v_aug[:, slot + sub, :],
                                            start=(kb == 0), stop=(kb == qb))
                            att = work.tile([P, GW], BF16, tag="att",
                                            name="att")
                            if attn_sub >= 5:
                                for sub in range(HPC):
                                    rec = small.tile([P, 1], F32, tag="rec",
                                                     name="rec")
                                    nc.vector.reciprocal(
                                        rec[:], pav[sub][:, HD:HD + 1])
                                    nc.vector.tensor_scalar_mul(
                                        att[:, sub * HD:(sub + 1) * HD],
                                        pav[sub][:, 0:HD], scalar1=rec[:])
                            else:
                                nc.vector.memset(att[:], 0.5)
                            nc.sync.dma_start(a2a_in[dst_c, dst_i], att[:])

            if active("a2a"):
                nc.gpsimd.collective_compute(
                    "AllToAll", OP.bypass,
                    replica_groups=[list(range(NCORES))],
                    ins=[a2a_in.opt()], outs=[a2a_out.opt()])

                att_tm = persist.tile([P, NBLK, D], BF16, tag="tm_buf")
                for r in range(NCORES):
                    for t in range(NBLK):
                        nc.sync.dma_start(
                            att_tm[:, t, r * GW:(r + 1) * GW], a2a_out[r, t])
                attT = persist.tile([P, DK, TOK], BF16, tag="fm_buf")
                for t in range(NBLK):
                    for d in range(DK):
                        nc.sync.dma_start(
                            attT[:, d, t * P:(t + 1) * P],
                            att_tm[:, t, d * P:(d + 1) * P],
                            transpose=True)

            if active("proj"):
                # ------------- stage D: proj + residual -> x2 -------------
                bproj_bc = persist.tile([P, D], F32)
                nc.sync.dma_start(bproj_bc[:], bcast_rows(bproj))
                wproj_sb = persist.tile([P, DK, D], BF16)
                nc.sync.dma_start(wproj_sb[:],
                                  wproj.rearrange("d p f -> p d f"))
                x2 = persist.tile([P, NBLK, D], F32)
                with tc.tile_pool(name="ps2", bufs=2, space="PSUM") as ps2:
                    for t in range(NBLK):
                        for n in range(D // 512):
                            ps = ps2.tile([P, 512], F32, tag="mm", name="ps")
                            for d in range(DK):
                                nc.tensor.matmul(
                                    ps[:], attT[:, d, t * P:(t + 1) * P],
                                    wproj_sb[:, d, n * 512:(n + 1) * 512],
                                    start=(d == 0), stop=(d == DK - 1))
                            sl = slice(n * 512, (n + 1) * 512)
                            nc.vector.tensor_tensor(
                                x2[:, t, sl], ps[:], x_res[:, t, sl], OP.add)
                            nc.vector.tensor_tensor(
                                x2[:, t, sl], x2[:, t, sl], bproj_bc[:, sl],
                                OP.add)

            if active("ln2"):
                # ---------------- stage E: LN2 -> h2T ----------------
                h2_tm = persist.tile([P, NBLK, D], BF16, tag="tm_buf")
                for t in range(NBLK):
                    st = small.tile([P, 2, 6], F32, tag="ln_st")
                    xg = x2[:, t, :].rearrange("p (s f) -> p s f", s=2)
                    for s in range(2):
                        nc.vector.bn_stats(out=st[:, s, :], in_=xg[:, s, :])
                    mv = small.tile([P, 2], F32, tag="ln_mv")
                    nc.vector.bn_aggr(out=mv[:], in_=st[:])
                    rstd = small.tile([P, 1], F32, tag="ln_rstd")
                    nc.scalar.activation(rstd[:], mv[:, 1:2], AF.Sqrt,
                                         bias=eps_t[:])
                    nc.vector.reciprocal(rstd[:], rstd[:])
                    nc.vector.tensor_scalar(
                        h2_tm[:, t, :], x2[:, t, :],
                        scalar1=mv[:, 0:1], scalar2=rstd[:],
                        op0=OP.subtract, op1=OP.mult)
                h2T = persist.tile([P, DK, TOK], BF16, tag="fm_buf")
                for t in range(NBLK):
                    for d in range(DK):
                        nc.sync.dma_start(
                            h2T[:, d, t * P:(t + 1) * P],
                            h2_tm[:, t, d * P:(d + 1) * P],
                            transpose=True)

            if active("ff1"):
                # ---------------- stage F1: FFN up + gelu ----------------
                b1_sb = persist.tile([P, NFF], F32)
                nc.sync.dma_start(b1_sb[:], b1e.rearrange("(c p) -> p c", p=P))
                gT = persist.tile([P, NFF, TOK], BF16, tag="big_buf")
                with tc.tile_pool(name="ps3", bufs=2, space="PSUM") as ps3:
                    for f in range(NFF):
                        ps = ps3.tile([P, TOK], F32, tag="mm", name="ps")
                        w1t = wstream.tile([P, DK, P], BF16, tag="w1_t",
                                           name="w1t")
                        nc.sync.dma_start(
                            w1t[:], w1[:, :, f * P:(f + 1) * P]
                            .rearrange("d p f -> p d f"))
                        for d in range(DK):
                            nc.tensor.matmul(
                                ps[:], w1t[:, d, :], h2T[:, d, :],
                                start=(d == 0), stop=(d == DK - 1))
                        nc.scalar.activation(gT[:, f, :], ps[:], AF.Gelu,
                                             bias=b1_sb[:, f:f + 1])

            if active("ff2"):
                # ---------------- stage F2: FFN down + residual ------------
                b2_bc = persist.tile([P, D], F32)
                nc.sync.dma_start(b2_bc[:], bcast_rows(b2))
                NOUT = D // 512
                with tc.tile_pool(name="ps4", bufs=1, space="PSUM") as ps4:
                    ps_out = [ps4.tile([P, 512], F32, tag=f"ff2_{i}",
                                       name=f"ff2_{i}")
                              for i in range(NBLK * NOUT)]
                    for f in range(NFF):
                        w2t = wstream.tile([P, D], BF16, tag="w2_t",
                                           name="w2t")
                        nc.sync.dma_start(w2t[:], w2[f])
                        for t in range(NBLK):
                            for n in range(NOUT):
                                nc.tensor.matmul(
                                    ps_out[t * NOUT + n][:],
                                    gT[:, f, t * P:(t + 1) * P],
                                    w2t[:, n * 512:(n + 1) * 512],
                                    start=(f == 0), stop=(f == NFF - 1))
                    for t in range(NBLK):
                        for n in range(NOUT):
                            sl = slice(n * 512, (n + 1) * 512)
                            o = work.tile([P, 512], F32, tag="out_sb",
                                          name="o")
                            nc.vector.tensor_tensor(
                                o[:], ps_out[t * NOUT + n][:], x2[:, t, sl],
                                OP.add)
                            nc.vector.tensor_tensor(o[:], o[:], b2_bc[:, sl],
                                                    OP.add)
                            nc.sync.dma_start(out[t][:, sl], o[:])
            else:
                # truncated build (debug): write deterministic junk to out
                for t in range(NBLK):
                    o = work.tile([P, D], F32, tag="dbg_out", name="o")
                    nc.vector.memset(o[:], 1.0 + t)
                    nc.sync.dma_start(out[t], o[:])

    nc.compile()
    meta = dict(T=T, B=B, D=D, H=H, FF=FF, NB=NB, own=own, HPC=HPC,
                NBLK=NBLK, TOK=TOK, DK=DK, NFF=NFF)
    return nc, meta


_BUILT = {}


def _get_built(T, B, stop_after=None, attn_sub=5):
    key = (T, B, stop_after, attn_sub)
    if key not in _BUILT:
        _BUILT[key] = build(T=T, B=B, stop_after=stop_after, attn_sub=attn_sub)
    return _BUILT[key]


def _prep_inputs(inputs, meta):
    """Host-side: fold LN params into weights, cast bf16, shard per core."""
    B, T, D = meta["B"], meta["T"], meta["D"]
    H, FF = meta["H"], meta["FF"]
    HD = D // H
    NB, NBLK, own = meta["NB"], meta["NBLK"], meta["own"]
    DK, NFF, HPC = meta["DK"], meta["NFF"], meta["HPC"]

    f32 = np.float32
    bf = ml_dtypes.bfloat16
    x = np.asarray(inputs["x"], f32)
    g1 = np.asarray(inputs["ln1_g"], f32)
    b1n = np.asarray(inputs["ln1_b"], f32)
    g2 = np.asarray(inputs["ln2_g"], f32)
    b2n = np.asarray(inputs["ln2_b"], f32)
    wq = np.asarray(inputs["wq"], f32)   # [H, D, HD]
    wk = np.asarray(inputs["wk"], f32)
    wv = np.asarray(inputs["wv"], f32)

    wq_f = wq * g1[None, :, None]
    wk_f = wk * g1[None, :, None]
    wv_f = wv * g1[None, :, None]
    bq = np.einsum("c,hcd->hd", b1n, wq)
    bk = np.einsum("c,hcd->hd", b1n, wk)
    bv = np.einsum("c,hcd->hd", b1n, wv)
    assert np.abs(bv).max() == 0.0, "nonzero folded V bias unsupported"

    WQ = wq_f.transpose(1, 0, 2).reshape(D, D)   # [D, H*HD] head-major cols
    WK = wk_f.transpose(1, 0, 2).reshape(D, D)
    WV = wv_f.transpose(1, 0, 2).reshape(D, D)
    GW = HPC * HD
    wqkv_cores, qkvb_cores = [], []
    for c in range(NCORES):
        cs = slice(c * GW, (c + 1) * GW)
        wcat = np.concatenate([WQ[:, cs], WK[:, cs], WV[:, cs]], axis=1)
        wqkv_cores.append(
            np.ascontiguousarray(wcat.reshape(DK, P, 3 * GW)).astype(bf))
        qb_ = np.concatenate([bq.reshape(-1)[cs], bk.reshape(-1)[cs],
                              bv.reshape(-1)[cs]]).astype(f32)
        qkvb_cores.append(np.ascontiguousarray(qb_))

    w_proj = np.asarray(inputs["w_proj"], f32)
    b_proj = np.ascontiguousarray(np.asarray(inputs["b_proj"], f32))
    w1f = np.asarray(inputs["w1"], f32)
    w1 = w1f * g2[:, None]
    b1e = (np.asarray(inputs["b1"], f32) + b2n @ w1f).astype(f32)
    w2 = np.asarray(inputs["w2"], f32)
    b2 = np.ascontiguousarray(np.asarray(inputs["b2"], f32))

    wproj_d = np.ascontiguousarray(w_proj.reshape(DK, P, D)).astype(bf)
    w1_d = np.ascontiguousarray(w1.reshape(DK, P, FF)).astype(bf)
    w2_d = np.ascontiguousarray(w2.reshape(NFF, P, D)).astype(bf)
    b1e = np.ascontiguousarray(b1e)

    in_maps = []
    for c in range(NCORES):
        xc = np.empty((NBLK, P, D), f32)
        for (b, qb), (cc, i) in own.items():
            if cc != c:
                continue
            xc[i] = x[b, qb * P:(qb + 1) * P, :]
        in_maps.append({
            "xin": xc,
            "wqkv": wqkv_cores[c],
            "qkvb": qkvb_cores[c],
            "wproj": wproj_d,
            "bproj": b_proj,
            "w1": w1_d,
            "b1e": b1e,
            "w2": w2_d,
            "b2": b2,
        })
    return in_maps


def _gather_output(results, meta):
    B, T, D = meta["B"], meta["T"], meta["D"]
    NB, NBLK, own = meta["NB"], meta["NBLK"], meta["own"]
    out = np.empty((B, T, D), np.float32)
    for (b, qb), (c, i) in own.items():
        out[b, qb * P:(qb + 1) * P, :] = results[c]["out"][i]
    return out


def run(inputs, T=2048, B=2, trace=False, stop_after=None, attn_sub=5,
        **spmd_kwargs):
    nc, meta = _get_built(T, B, stop_after, attn_sub)
    in_maps = _prep_inputs(inputs, meta)
    res = run_bass_kernel_spmd(
        nc, in_maps, core_ids=list(range(NCORES)), trace=trace, **spmd_kwargs)
    return _gather_output(res.results, meta), res


def kernel(**inputs):
    out, _ = run(inputs, T=2048, B=2, trace=False)
    return out


# revision 14
# speedup vs baseline: 1.4053x; 1.4053x over previous
"""Trainium2 Bass kernel for a dense transformer block (pre-LN, causal MHA + FFN).

Reference shapes: x [B=2, T=2048, D=1024], H=16 heads, HD=64, FF=4096, f32.

Sharding (8 NeuronCores, SPMD single program):
  - Tokens (B*T = 32 blocks of 128) are owned by cores: core c owns blocks
    (c, 15-c) of each batch -> 512 tokens/core. LN1/LN2, residuals, proj and
    FFN are pure data-parallel over tokens.
  - Attention is head-sharded (2 heads/core over ALL tokens) so the causal
    loop structure is identical on every core. Two cheap collectives glue the
    shardings: AllGather of the LN1 output (feature-major, bf16, 1MB/rank)
    before QKV, and AllToAll of the attention output (1MB/rank) after.
  - Weights are cast to bf16 on host; LN gains/biases are folded into the
    adjacent weight matrices on host. Matmuls run in bf16 (fp32 accumulate),
    the residual stream stays f32 on device.
"""
import numpy as np
import ml_dtypes

import concourse.bass as bass
from concourse import bacc
import concourse.tile as tile
import concourse.mybir as mybir
from concourse.bass_utils import run_bass_kernel_spmd
from concourse.masks import make_identity

F32 = mybir.dt.float32
BF16 = mybir.dt.bfloat16
AF = mybir.ActivationFunctionType
OP = mybir.AluOpType

P = 128
NCORES = 8

STAGES = ["ln1", "ag", "qkv", "attn", "a2a", "proj", "ln2", "ff1", "ff2"]


def _owner_map(b_total, nb):
    """(batch, block) -> (core, local block idx). Full size pairs block i with
    nb-1-i on one core so causal attention work is balanced."""
    own = {}
    if nb == 2 * NCORES:
        for b in range(b_total):
            for qb in range(nb):
                c = qb if qb < NCORES else nb - 1 - qb
                own[(b, qb)] = (c, 2 * b + (0 if qb < NCORES else 1))
    else:
        for b in range(b_total):
            for qb in range(nb):
                inst = b * nb + qb
                own[(b, qb)] = (inst % NCORES, inst // NCORES)
    return own


def build(T=2048, B=2, D=1024, H=16, FF=4096, eps=1e-5, stop_after=None, attn_sub=5):
    HD = D // H
    NB = T // P                    # blocks per batch
    HPC = H // NCORES              # heads per core
    NBLK = B * NB // NCORES        # owned token blocks per core
    TOK = NBLK * P                 # owned tokens per core
    TQ = NCORES * TOK              # all tokens (B*T)
    DK = D // P                    # d_model chunks
    NFF = FF // P                  # ff chunks
    NKB = B * NB                   # total key blocks
    own = _owner_map(B, NB)
    GW = HPC * HD                  # my attention feature width (128)
    assert GW == P

    if stop_after is None:
        stop_after = "ff2"
    assert stop_after in STAGES

    def active(stage):
        return STAGES.index(stage) <= STAGES.index(stop_after)

    def blkidx(b, qb):
        return own[(b, qb)]

    def tokcol(b, qb):
        c, i = blkidx(b, qb)
        return c * TOK + i * P

    nc = bacc.Bacc("TRN2", num_devices=NCORES)

    # ---- DRAM I/O (per core) ----
    xin = nc.dram_tensor("xin", [NBLK, P, D], F32, kind="ExternalInput").ap()
    wqkv = nc.dram_tensor("wqkv", [DK, P, 3 * GW], BF16, kind="ExternalInput").ap()
    qkvb = nc.dram_tensor("qkvb", [3 * GW], F32, kind="ExternalInput").ap()
    wproj = nc.dram_tensor("wproj", [DK, P, D], BF16, kind="ExternalInput").ap()
    bproj = nc.dram_tensor("bproj", [D], F32, kind="ExternalInput").ap()
    w1 = nc.dram_tensor("w1", [DK, P, FF], BF16, kind="ExternalInput").ap()
    b1e = nc.dram_tensor("b1e", [FF], F32, kind="ExternalInput").ap()
    w2 = nc.dram_tensor("w2", [NFF, P, D], BF16, kind="ExternalInput").ap()
    b2 = nc.dram_tensor("b2", [D], F32, kind="ExternalInput").ap()
    out = nc.dram_tensor("out", [NBLK, P, D], F32, kind="ExternalOutput").ap()

    def bcast_rows(vec_ap):
        return bass.AP(tensor=vec_ap.tensor, offset=vec_ap.offset,
                       ap=[[0, P]] + list(vec_ap.ap))

    with tile.TileContext(nc) as tc:
        with (
            tc.tile_pool(name="persist", bufs=1) as persist,
            tc.tile_pool(name="dram", bufs=1, space="DRAM") as dram,
            tc.tile_pool(name="wstream", bufs=3) as wstream,
            tc.tile_pool(name="work", bufs=3) as work,
            tc.tile_pool(name="small", bufs=4) as small,
        ):
            ident = persist.tile([P, P], BF16)
            make_identity(nc, ident[:])
            tril = persist.tile([P, P], BF16)
            nc.vector.memset(tril[:], 1.0)
            nc.gpsimd.affine_select(
                out=tril[:], in_=tril[:], compare_op=OP.is_ge,
                fill=0.0, base=0, channel_multiplier=-1, pattern=[[1, P]])

            def pe_transpose(pool, dst_slice, src_slice):
                pt = pool.tile([P, P], BF16, tag="tr", name="pt")
                nc.tensor.transpose(pt[:], src_slice, ident[:])
                nc.vector.tensor_copy(dst_slice, pt[:])

            # ---------------- stage A: LN1 over own tokens ----------------
            x_res = persist.tile([P, NBLK, D], F32)
            h_tm = persist.tile([P, NBLK, D], BF16, tag="tm_buf")
            eps_t = persist.tile([P, 1], F32)
            nc.vector.memset(eps_t[:], eps)
            for t in range(NBLK):
                nc.sync.dma_start(x_res[:, t, :], xin[t])
                st = small.tile([P, 2, 6], F32, tag="ln_st")
                xg = x_res[:, t, :].rearrange("p (s f) -> p s f", s=2)
                for s in range(2):
                    nc.vector.bn_stats(out=st[:, s, :], in_=xg[:, s, :])
                mv = small.tile([P, 2], F32, tag="ln_mv")
                nc.vector.bn_aggr(out=mv[:], in_=st[:])
                rstd = small.tile([P, 1], F32, tag="ln_rstd")
                nc.scalar.activation(rstd[:], mv[:, 1:2], AF.Sqrt, bias=eps_t[:])
                nc.vector.reciprocal(rstd[:], rstd[:])
                nc.vector.tensor_scalar(
                    h_tm[:, t, :], x_res[:, t, :],
                    scalar1=mv[:, 0:1], scalar2=rstd[:],
                    op0=OP.subtract, op1=OP.mult)

            # transpose h -> feature-major [P, DK, TOK] via PE transpose
            hT_own = persist.tile([P, DK, TOK], BF16, tag="fm_buf")
            with tc.tile_pool(name="ps_tr1", bufs=4, space="PSUM") as ps_tr1:
                for t in range(NBLK):
                    for d in range(DK):
                        pe_transpose(ps_tr1,
                                     hT_own[:, d, t * P:(t + 1) * P],
                                     h_tm[:, t, d * P:(d + 1) * P])

            if active("ag"):
                # ---------------- AllGather hT ----------------
                ag_in = dram.tile([DK, P, TOK], BF16)
                ag_out = dram.tile([NCORES, DK, P, TOK], BF16,
                   addr_space="Shared")
                for d in range(DK):
                    nc.sync.dma_start(ag_in[d], hT_own[:, d, :])
                nc.gpsimd.collective_compute(
                    "AllGather", OP.bypass,
                    replica_groups=[list(range(NCORES))],
                    ins=[ag_in.opt()], outs=[ag_out.opt()])

                hT = persist.tile([P, DK, TQ], BF16, tag="big_buf")
                for r in range(NCORES):
                    for d in range(DK):
                        nc.sync.dma_start(
                            hT[:, d, r * TOK:(r + 1) * TOK], ag_out[r, d])

            if active("qkv"):
                # ---------------- stage B: QKV for my heads ----------------
                qT = persist.tile([P, TQ], BF16)
                kT = persist.tile([P, TQ], BF16)
                qkvb_sb = persist.tile([P, 3], F32)
                nc.sync.dma_start(qkvb_sb[:],
                                  qkvb.rearrange("(g p) -> p g", p=P))
                wqkv_sb = persist.tile([P, DK, 3 * GW], BF16)
                nc.sync.dma_start(wqkv_sb[:], wqkv.rearrange("d p f -> p d f"))
                NT = TQ // 512
                with tc.tile_pool(name="ps1", bufs=2, space="PSUM") as ps1:
                    for g, dst in ((0, qT), (1, kT)):
                        for nt in range(NT):
                            ps = ps1.tile([P, 512], F32, tag="mm", name="ps")
                            for d in range(DK):
                                nc.tensor.matmul(
                                    ps[:], wqkv_sb[:, d, g * P:(g + 1) * P],
                                    hT[:, d, nt * 512:(nt + 1) * 512],
                                    start=(d == 0), stop=(d == DK - 1))
                            nc.vector.tensor_scalar_add(
                                dst[:, nt * 512:(nt + 1) * 512], ps[:],
                                scalar1=qkvb_sb[:, g:g + 1])

                    # V token-major with fused ones column
                    # (folded LN1 bias for V must be zero; host asserts)
                    v_aug = persist.tile([P, NKB * HPC, HD + 1], BF16)
                    nc.vector.memset(v_aug[:, :, HD:HD + 1], 1.0)
                    for b in range(B):
                        for kb in range(NB):
                            col = tokcol(b, kb)
                            slot = (b * NB + kb) * HPC
                            ps = ps1.tile([P, 512], F32, tag="mm", name="ps")
                            for d in range(DK):
                                nc.tensor.matmul(
                                    ps[:, 0:GW], hT[:, d, col:col + P],
                                    wqkv_sb[:, d, 2 * P:3 * P],
                                    start=(d == 0), stop=(d == DK - 1))
                            for sub in range(HPC):
                                nc.vector.tensor_copy(
                                    v_aug[:, slot + sub, 0:HD],
                                    ps[:, sub * HD:(sub + 1) * HD])

            if active("attn"):
                # ------------- stage C: attention (my HPC heads) -----------
                a2a_in = dram.tile([NCORES, NBLK, P, GW], BF16)
                a2a_out = dram.tile([NCORES, NBLK, P, GW], BF16)

                # q-groups: a core's owned blocks of one batch are adjacent
                # in the gathered token order; process each group's causal
                # range with merged score tiles and 2-subhead-batched exp.
                qgroups = []
                for b in range(B):
                    for c in range(NCORES):
                        mem = sorted(
                            (i, qb) for (bb, qb), (cc, i) in own.items()
                            if bb == b and cc == c)
                        qbs = [qb for _, qb in mem]
                        if not qbs:
                            continue
                        assert all(q1 < q2 for q1, q2 in zip(qbs, qbs[1:]))
                        col0 = tokcol(b, qbs[0])
                        for j, qb in enumerate(qbs):
                            assert tokcol(b, qb) == col0 + j * P
                        qgroups.append((b, qbs, col0))

                with (
                    tc.tile_pool(name="ps_s", bufs=2, space="PSUM") as ps_sp,
                    tc.tile_pool(name="ps_av", bufs=1, space="PSUM") as ps_avp,
                ):
                    for b, qbs, col0 in qgroups:
                        M = len(qbs)
                        pav = [[ps_avp.tile([P, 512], F32, tag=f"av{m}_{s}",
                                            name=f"pav{m}_{s}")[:, 0:HD + 1]
                                for s in range(HPC)] for m in range(M)]
                        for kb in range(qbs[-1] + 1):
                            kcol = tokcol(b, kb)
                            fa = next(m for m in range(M) if qbs[m] >= kb)
                            qoff = col0 + fa * P
                            N = (M - fa) * P
                            ps2 = ps_sp.tile([P, HPC, 512], F32,
                                             tag="score", name="ps2")
                            if attn_sub < 2:
                                continue
                            for sub in range(HPC):
                                nc.tensor.matmul(
                                    ps2[:, sub, 0:N],
                                    kT[sub * HD:(sub + 1) * HD,
                                       kcol:kcol + P],
                                    qT[sub * HD:(sub + 1) * HD,
                                       qoff:qoff + N],
                                    start=True, stop=True)
                            p_sb = work.tile([P, HPC, 2 * P], BF16,
                                             tag="p_sb", name="p_sb")
                            nc.scalar.activation(
                                p_sb[:, :, 0:N], ps2[:, :, 0:N], AF.Exp,
                                scale=float(HD) ** -0.5)
                            if attn_sub >= 3:
                                for m in range(fa, M):
                                    if qbs[m] != kb:
                                        continue
                                    moff = (m - fa) * P
                                    for sub in range(HPC):
                                        nc.vector.tensor_tensor(
                                            p_sb[:, sub, moff:moff + P],
                                            p_sb[:, sub, moff:moff + P],
                                            tril[:], OP.mult)
                            slot = (b * NB + kb) * HPC
                            if attn_sub >= 4:
                                for m in range(fa, M):
                                    moff = (m - fa) * P
                                    for sub in range(HPC):
                                        nc.tensor.matmul(
                                            pav[m][sub][:],
                                            p_sb[:, sub, moff:moff + P],
                                            v_aug[:, slot + sub, :],
                                            start=(kb == 0),
                                            stop=(kb == qbs[m]))
                        for m in range(M):
                            dst_c, dst_i = blkidx(b, qbs[m])
                            att = work.tile([P, GW], BF16, tag="att",
                                            name="att")
                            if attn_sub >= 5:
                                for sub in range(HPC):
                                    rec = small.tile([P, 1], F32, tag="rec",
                                                     name="rec")
                                    nc.vector.reciprocal(
                                        rec[:], pav[m][sub][:, HD:HD + 1])
                                    nc.vector.tensor_scalar_mul(
                                        att[:, sub * HD:(sub + 1) * HD],
                                        pav[m][sub][:, 0:HD], scalar1=rec[:])
                            else:
                                nc.vector.memset(att[:], 0.5)
                            nc.sync.dma_start(a2a_in[dst_c, dst_i], att[:])

            if active("a2a"):
                nc.gpsimd.collective_compute(
                    "AllToAll", OP.bypass,
                    replica_groups=[list(range(NCORES))],
                    ins=[a2a_in.opt()], outs=[a2a_out.opt()])

                att_tm = persist.tile([P, NBLK, D], BF16, tag="tm_buf")
                for r in range(NCORES):
                    for t in range(NBLK):
                        nc.sync.dma_start(
                            att_tm[:, t, r * GW:(r + 1) * GW], a2a_out[r, t])
                attT = persist.tile([P, DK, TOK], BF16, tag="fm_buf")
                with tc.tile_pool(name="ps_tr2", bufs=4,
                                  space="PSUM") as ps_tr2:
                    for t in range(NBLK):
                        for d in range(DK):
                            pe_transpose(ps_tr2,
                                         attT[:, d, t * P:(t + 1) * P],
                                         att_tm[:, t, d * P:(d + 1) * P])

            if active("proj"):
                # ------------- stage D: proj + residual -> x2 -------------
                bproj_bc = persist.tile([P, D], F32)
                nc.sync.dma_start(bproj_bc[:], bcast_rows(bproj))
                wproj_sb = persist.tile([P, DK, D], BF16)
                nc.sync.dma_start(wproj_sb[:],
                                  wproj.rearrange("d p f -> p d f"))
                x2 = persist.tile([P, NBLK, D], F32)
                with tc.tile_pool(name="ps2", bufs=2, space="PSUM") as ps2:
                    for t in range(NBLK):
                        for n in range(D // 512):
                            ps = ps2.tile([P, 512], F32, tag="mm", name="ps")
                            for d in range(DK):
                                nc.tensor.matmul(
                                    ps[:], attT[:, d, t * P:(t + 1) * P],
                                    wproj_sb[:, d, n * 512:(n + 1) * 512],
                                    start=(d == 0), stop=(d == DK - 1))
                            sl = slice(n * 512, (n + 1) * 512)
                            nc.vector.tensor_tensor(
                                x2[:, t, sl], ps[:], x_res[:, t, sl], OP.add)
                            nc.vector.tensor_tensor(
                                x2[:, t, sl], x2[:, t, sl], bproj_bc[:, sl],
                                OP.add)

            if active("ln2"):
                # ---------------- stage E: LN2 -> h2T ----------------
                h2_tm = persist.tile([P, NBLK, D], BF16, tag="tm_buf")
                for t in range(NBLK):
                    st = small.tile([P, 2, 6], F32, tag="ln_st")
                    xg = x2[:, t, :].rearrange("p (s f) -> p s f", s=2)
                    for s in range(2):
                        nc.vector.bn_stats(out=st[:, s, :], in_=xg[:, s, :])
                    mv = small.tile([P, 2], F32, tag="ln_mv")
                    nc.vector.bn_aggr(out=mv[:], in_=st[:])
                    rstd = small.tile([P, 1], F32, tag="ln_rstd")
                    nc.scalar.activation(rstd[:], mv[:, 1:2], AF.Sqrt,
                                         bias=eps_t[:])
                    nc.vector.reciprocal(rstd[:], rstd[:])
                    nc.vector.tensor_scalar(
                        h2_tm[:, t, :], x2[:, t, :],
                        scalar1=mv[:, 0:1], scalar2=rstd[:],
                        op0=OP.subtract, op1=OP.mult)
                h2T = persist.tile([P, DK, TOK], BF16, tag="fm_buf")
                with tc.tile_pool(name="ps_tr3", bufs=4,
                                  space="PSUM") as ps_tr3:
                    for t in range(NBLK):
                        for d in range(DK):
                            pe_transpose(ps_tr3,
                                         h2T[:, d, t * P:(t + 1) * P],
                                         h2_tm[:, t, d * P:(d + 1) * P])

            if active("ff1"):
                # ---------------- stage F1: FFN up + gelu ----------------
                b1_sb = persist.tile([P, NFF], F32)
                nc.sync.dma_start(b1_sb[:], b1e.rearrange("(c p) -> p c", p=P))
                gT = persist.tile([P, NFF, TOK], BF16, tag="big_buf")
                with tc.tile_pool(name="ps3", bufs=2, space="PSUM") as ps3:
                    for f in range(NFF):
                        ps = ps3.tile([P, TOK], F32, tag="mm", name="ps")
                        w1t = wstream.tile([P, DK, P], BF16, tag="w1_t",
                                           name="w1t")
                        nc.sync.dma_start(
                            w1t[:], w1[:, :, f * P:(f + 1) * P]
                            .rearrange("d p f -> p d f"))
                        for d in range(DK):
                            nc.tensor.matmul(
                                ps[:], w1t[:, d, :], h2T[:, d, :],
                                start=(d == 0), stop=(d == DK - 1))
                        nc.scalar.activation(gT[:, f, :], ps[:], AF.Gelu,
                                             bias=b1_sb[:, f:f + 1])

            if active("ff2"):
                # ---------------- stage F2: FFN down + residual ------------
                b2_bc = persist.tile([P, D], F32)
                nc.sync.dma_start(b2_bc[:], bcast_rows(b2))
                NOUT = D // 512
                with tc.tile_pool(name="ps4", bufs=1, space="PSUM") as ps4:
                    ps_out = [ps4.tile([P, 512], F32, tag=f"ff2_{i}",
                                       name=f"ff2_{i}")
                              for i in range(NBLK * NOUT)]
                    for f in range(NFF):
                        w2t = wstream.tile([P, D], BF16, tag="w2_t",
                                           name="w2t")
                        nc.sync.dma_start(w2t[:], w2[f])
                        for t in range(NBLK):
                            for n in range(NOUT):
                                nc.tensor.matmul(
                                    ps_out[t * NOUT + n][:],
                                    gT[:, f, t * P:(t + 1) * P],
                                    w2t[:, n * 512:(n + 1) * 512],
                                    start=(f == 0), stop=(f == NFF - 1))
                    for t in range(NBLK):
                        for n in range(NOUT):
                            sl = slice(n * 512, (n + 1) * 512)
                            o = work.tile([P, 512], F32, tag="out_sb",
                                          name="o")
                            nc.vector.tensor_tensor(
                                o[:], ps_out[t * NOUT + n][:], x2[:, t, sl],
                                OP.add)
                            nc.vector.tensor_tensor(o[:], o[:], b2_bc[:, sl],
                                                    OP.add)
                            nc.sync.dma_start(out[t][:, sl], o[:])
            else:
                # truncated build (debug): write deterministic junk to out
                for t in range(NBLK):
                    o = work.tile([P, D], F32, tag="dbg_out", name="o")
                    nc.vector.memset(o[:], 1.0 + t)
                    nc.sync.dma_start(out[t], o[:])

    nc.compile()
    meta = dict(T=T, B=B, D=D, H=H, FF=FF, NB=NB, own=own, HPC=HPC,
                NBLK=NBLK, TOK=TOK, DK=DK, NFF=NFF)
    return nc, meta


_BUILT = {}


def _get_built(T, B, stop_after=None, attn_sub=5):
    key = (T, B, stop_after, attn_sub)
    if key not in _BUILT:
        _BUILT[key] = build(T=T, B=B, stop_after=stop_after, attn_sub=attn_sub)
    return _BUILT[key]


def _prep_inputs(inputs, meta):
    """Host-side: fold LN params into weights, cast bf16, shard per core."""
    B, T, D = meta["B"], meta["T"], meta["D"]
    H, FF = meta["H"], meta["FF"]
    HD = D // H
    NB, NBLK, own = meta["NB"], meta["NBLK"], meta["own"]
    DK, NFF, HPC = meta["DK"], meta["NFF"], meta["HPC"]

    f32 = np.float32
    bf = ml_dtypes.bfloat16
    x = np.asarray(inputs["x"], f32)
    g1 = np.asarray(inputs["ln1_g"], f32)
    b1n = np.asarray(inputs["ln1_b"], f32)
    g2 = np.asarray(inputs["ln2_g"], f32)
    b2n = np.asarray(inputs["ln2_b"], f32)
    wq = np.asarray(inputs["wq"], f32)   # [H, D, HD]
    wk = np.asarray(inputs["wk"], f32)
    wv = np.asarray(inputs["wv"], f32)

    wq_f = wq * g1[None, :, None]
    wk_f = wk * g1[None, :, None]
    wv_f = wv * g1[None, :, None]
    bq = np.einsum("c,hcd->hd", b1n, wq)
    bk = np.einsum("c,hcd->hd", b1n, wk)
    bv = np.einsum("c,hcd->hd", b1n, wv)
    assert np.abs(bv).max() == 0.0, "nonzero folded V bias unsupported"

    WQ = wq_f.transpose(1, 0, 2).reshape(D, D)   # [D, H*HD] head-major cols
    WK = wk_f.transpose(1, 0, 2).reshape(D, D)
    WV = wv_f.transpose(1, 0, 2).reshape(D, D)
    GW = HPC * HD
    wqkv_cores, qkvb_cores = [], []
    for c in range(NCORES):
        cs = slice(c * GW, (c + 1) * GW)
        wcat = np.concatenate([WQ[:, cs], WK[:, cs], WV[:, cs]], axis=1)
        wqkv_cores.append(
            np.ascontiguousarray(wcat.reshape(DK, P, 3 * GW)).astype(bf))
        qb_ = np.concatenate([bq.reshape(-1)[cs], bk.reshape(-1)[cs],
                              bv.reshape(-1)[cs]]).astype(f32)
        qkvb_cores.append(np.ascontiguousarray(qb_))

    w_proj = np.asarray(inputs["w_proj"], f32)
    b_proj = np.ascontiguousarray(np.asarray(inputs["b_proj"], f32))
    w1f = np.asarray(inputs["w1"], f32)
    w1 = w1f * g2[:, None]
    b1e = (np.asarray(inputs["b1"], f32) + b2n @ w1f).astype(f32)
    w2 = np.asarray(inputs["w2"], f32)
    b2 = np.ascontiguousarray(np.asarray(inputs["b2"], f32))

    wproj_d = np.ascontiguousarray(w_proj.reshape(DK, P, D)).astype(bf)
    w1_d = np.ascontiguousarray(w1.reshape(DK, P, FF)).astype(bf)
    w2_d = np.ascontiguousarray(w2.reshape(NFF, P, D)).astype(bf)
    b1e = np.ascontiguousarray(b1e)

    in_maps = []
    for c in range(NCORES):
        xc = np.empty((NBLK, P, D), f32)
        for (b, qb), (cc, i) in own.items():
            if cc != c:
                continue
            xc[i] = x[b, qb * P:(qb + 1) * P, :]
        in_maps.append({
            "xin": xc,
            "wqkv": wqkv_cores[c],
            "qkvb": qkvb_cores[c],
            "wproj": wproj_d,
            "bproj": b_proj,
            "w1": w1_d,
            "b1e": b1e,
            "w2": w2_d,
            "b2": b2,
        })
    return in_maps


def _gather_output(results, meta):
    B, T, D = meta["B"], meta["T"], meta["D"]
    NB, NBLK, own = meta["NB"], meta["NBLK"], meta["own"]
    out = np.empty((B, T, D), np.float32)
    for (b, qb), (c, i) in own.items():
        out[b, qb * P:(qb + 1) * P, :] = results[c]["out"][i]
    return out


def run(inputs, T=2048, B=2, trace=False, stop_after=None, attn_sub=5,
        **spmd_kwargs):
    nc, meta = _get_built(T, B, stop_after, attn_sub)
    in_maps = _prep_inputs(inputs, meta)
    res = run_bass_kernel_spmd(
        nc, in_maps, core_ids=list(range(NCORES)), trace=trace, **spmd_kwargs)
    return _gather_output(res.results, meta), res


def kernel(**inputs):
    out, _ = run(inputs, T=2048, B=2, trace=False)
    return out


# revision 19
# speedup vs baseline: 1.4326x; 1.0194x over previous
"""Trainium2 Bass kernel for a dense transformer block (pre-LN, causal MHA + FFN).

Reference shapes: x [B=2, T=2048, D=1024], H=16 heads, HD=64, FF=4096, f32.

Sharding (8 NeuronCores, SPMD single program):
  - Tokens (B*T = 32 blocks of 128) are owned by cores: core c owns blocks
    (c, 15-c) of each batch -> 512 tokens/core. LN1/LN2, residuals, proj and
    FFN are pure data-parallel over tokens.
  - Attention is head-sharded (2 heads/core over ALL tokens) so the causal
    loop structure is identical on every core. Two cheap collectives glue the
    shardings: AllGather of the LN1 output (feature-major, bf16, 1MB/rank)
    before QKV, and AllToAll of the attention output (1MB/rank) after.
  - Weights are cast to bf16 on host; LN gains/biases are folded into the
    adjacent weight matrices on host. Matmuls run in bf16 (fp32 accumulate),
    the residual stream stays f32 on device.
"""
import numpy as np
import ml_dtypes

import concourse.bass as bass
from concourse import bacc
import concourse.tile as tile
import concourse.mybir as mybir
from concourse.bass_utils import run_bass_kernel_spmd
from concourse.masks import make_identity

F32 = mybir.dt.float32
BF16 = mybir.dt.bfloat16
AF = mybir.ActivationFunctionType
OP = mybir.AluOpType

P = 128
NCORES = 8

STAGES = ["ln1", "ag", "qkv", "attn", "a2a", "proj", "ln2", "ff1", "ff2"]


def _owner_map(b_total, nb):
    """(batch, block) -> (core, local block idx). Full size pairs block i with
    nb-1-i on one core so causal attention work is balanced."""
    own = {}
    if nb == 2 * NCORES:
        for b in range(b_total):
            for qb in range(nb):
                c = qb if qb < NCORES else nb - 1 - qb
                own[(b, qb)] = (c, 2 * b + (0 if qb < NCORES else 1))
    else:
        for b in range(b_total):
            for qb in range(nb):
                inst = b * nb + qb
                own[(b, qb)] = (inst % NCORES, inst // NCORES)
    return own


def build(T=2048, B=2, D=1024, H=16, FF=4096, eps=1e-5, stop_after=None, attn_sub=5):
    HD = D // H
    NB = T // P                    # blocks per batch
    HPC = H // NCORES              # heads per core
    NBLK = B * NB // NCORES        # owned token blocks per core
    TOK = NBLK * P                 # owned tokens per core
    TQ = NCORES * TOK              # all tokens (B*T)
    DK = D // P                    # d_model chunks
    NFF = FF // P                  # ff chunks
    NKB = B * NB                   # total key blocks
    own = _owner_map(B, NB)
    GW = HPC * HD                  # my attention feature width (128)
    assert GW == P

    if stop_after is None:
        stop_after = "ff2"
    assert stop_after in STAGES

    def active(stage):
        return STAGES.index(stage) <= STAGES.index(stop_after)

    def blkidx(b, qb):
        return own[(b, qb)]

    def tokcol(b, qb):
        c, i = blkidx(b, qb)
        return c * TOK + i * P

    nc = bacc.Bacc("TRN2", num_devices=NCORES)

    # ---- DRAM I/O (per core) ----
    xin = nc.dram_tensor("xin", [NBLK, P, D], F32, kind="ExternalInput").ap()
    wqkv = nc.dram_tensor("wqkv", [DK, P, 3 * GW], BF16, kind="ExternalInput").ap()
    qkvb = nc.dram_tensor("qkvb", [3 * GW], F32, kind="ExternalInput").ap()
    wproj = nc.dram_tensor("wproj", [DK, P, D], BF16, kind="ExternalInput").ap()
    bproj = nc.dram_tensor("bproj", [D], F32, kind="ExternalInput").ap()
    w1 = nc.dram_tensor("w1", [DK, P, FF], BF16, kind="ExternalInput").ap()
    b1e = nc.dram_tensor("b1e", [FF], F32, kind="ExternalInput").ap()
    w2 = nc.dram_tensor("w2", [NFF, P, D], BF16, kind="ExternalInput").ap()
    b2 = nc.dram_tensor("b2", [D], F32, kind="ExternalInput").ap()
    out = nc.dram_tensor("out", [NBLK, P, D], F32, kind="ExternalOutput").ap()

    def bcast_rows(vec_ap):
        return bass.AP(tensor=vec_ap.tensor, offset=vec_ap.offset,
                       ap=[[0, P]] + list(vec_ap.ap))

    with tile.TileContext(nc) as tc:
        with (
            tc.tile_pool(name="persist", bufs=1) as persist,
            tc.tile_pool(name="dram", bufs=1, space="DRAM") as dram,
            tc.tile_pool(name="wstream", bufs=3) as wstream,
            tc.tile_pool(name="work", bufs=3) as work,
            tc.tile_pool(name="small", bufs=4) as small,
        ):
            ident = persist.tile([P, P], BF16)
            make_identity(nc, ident[:])
            tril = persist.tile([P, P], BF16)
            nc.vector.memset(tril[:], 1.0)
            nc.gpsimd.affine_select(
                out=tril[:], in_=tril[:], compare_op=OP.is_ge,
                fill=0.0, base=0, channel_multiplier=-1, pattern=[[1, P]])

            def pe_transpose(pool, dst_slice, src_slice):
                pt = pool.tile([P, P], BF16, tag="tr", name="pt")
                nc.tensor.transpose(pt[:], src_slice, ident[:])
                nc.vector.tensor_copy(dst_slice, pt[:])

            # ---------------- stage A: LN1 over own tokens ----------------
            x_res = persist.tile([P, NBLK, D], F32)
            h_tm = persist.tile([P, NBLK, D], BF16, tag="tm_buf")
            eps_t = persist.tile([P, 1], F32)
            nc.vector.memset(eps_t[:], eps)
            for t in range(NBLK):
                nc.sync.dma_start(x_res[:, t, :], xin[t])
                st = small.tile([P, 2, 6], F32, tag="ln_st")
                xg = x_res[:, t, :].rearrange("p (s f) -> p s f", s=2)
                for s in range(2):
                    nc.vector.bn_stats(out=st[:, s, :], in_=xg[:, s, :])
                mv = small.tile([P, 2], F32, tag="ln_mv")
                nc.vector.bn_aggr(out=mv[:], in_=st[:])
                rstd = small.tile([P, 1], F32, tag="ln_rstd")
                nc.scalar.activation(rstd[:], mv[:, 1:2], AF.Sqrt, bias=eps_t[:])
                nc.vector.reciprocal(rstd[:], rstd[:])
                nc.vector.tensor_scalar(
                    h_tm[:, t, :], x_res[:, t, :],
                    scalar1=mv[:, 0:1], scalar2=rstd[:],
                    op0=OP.subtract, op1=OP.mult)

            # transpose h -> feature-major [P, DK, TOK] via PE transpose
            hT_own = persist.tile([P, DK, TOK], BF16, tag="fm_buf")
            with tc.tile_pool(name="ps_tr1", bufs=4, space="PSUM") as ps_tr1:
                for d in range(DK):
                    for t in range(NBLK):
                        pe_transpose(ps_tr1,
                                     hT_own[:, d, t * P:(t + 1) * P],
                                     h_tm[:, t, d * P:(d + 1) * P])

            if active("ag"):
                # ---------------- AllGather hT ----------------
                ag_in = dram.tile([DK, P, TOK], BF16)
                ag_out = dram.tile([NCORES, DK, P, TOK], BF16,
                                   addr_space="Shared")
                for d in range(DK):
                    nc.sync.dma_start(ag_in[d], hT_own[:, d, :])
                nc.gpsimd.collective_compute(
                    "AllGather", OP.bypass,
                    replica_groups=[list(range(NCORES))],
                    ins=[ag_in.opt()], outs=[ag_out.opt()])
                hT = persist.tile([P, DK, TQ], BF16, tag="big_buf")
                for r in range(NCORES):
                    for d in range(DK):
                        nc.sync.dma_start(
                            hT[:, d, r * TOK:(r + 1) * TOK], ag_out[r, d])

            if active("qkv"):
                # ---------------- stage B: QKV for my heads ----------------
                qT = persist.tile([P, TQ], BF16)
                kT = persist.tile([P, TQ], BF16)
                qkvb_sb = persist.tile([P, 3], F32)
                nc.scalar.dma_start(qkvb_sb[:],
                                  qkvb.rearrange("(g p) -> p g", p=P))
                wqkv_sb = persist.tile([P, DK, 3 * GW], BF16)
                nc.scalar.dma_start(wqkv_sb[:], wqkv.rearrange("d p f -> p d f"))
                NT = TQ // 512
                with tc.tile_pool(name="ps1", bufs=2, space="PSUM") as ps1:
                    for g, dst in ((0, qT), (1, kT)):
                        for nt in range(NT):
                            ps = ps1.tile([P, 512], F32, tag="mm", name="ps")
                            for d in range(DK):
                                nc.tensor.matmul(
                                    ps[:], wqkv_sb[:, d, g * P:(g + 1) * P],
                                    hT[:, d, nt * 512:(nt + 1) * 512],
                                    start=(d == 0), stop=(d == DK - 1))
                            nc.vector.tensor_scalar_add(
                                dst[:, nt * 512:(nt + 1) * 512], ps[:],
                                scalar1=qkvb_sb[:, g:g + 1])

                    # V token-major with fused ones column
                    # (folded LN1 bias for V must be zero; host asserts)
                    v_aug = persist.tile([P, NKB * HPC, HD + 1], BF16)
                    nc.vector.memset(v_aug[:, :, HD:HD + 1], 1.0)
                    for b in range(B):
                        for kb in range(NB):
                            col = tokcol(b, kb)
                            slot = (b * NB + kb) * HPC
                            ps = ps1.tile([P, 512], F32, tag="mm", name="ps")
                            for d in range(DK):
                                nc.tensor.matmul(
                                    ps[:, 0:GW], hT[:, d, col:col + P],
                                    wqkv_sb[:, d, 2 * P:3 * P],
                                    start=(d == 0), stop=(d == DK - 1))
                            for sub in range(HPC):
                                nc.vector.tensor_copy(
                                    v_aug[:, slot + sub, 0:HD],
                                    ps[:, sub * HD:(sub + 1) * HD])

            if active("attn"):
                # ------------- stage C: attention (my HPC heads) -----------
                # Per-batch AllToAll at full size so batch-0's A2A + attT
                # assembly overlap batch-1's attention compute.
                PIPE = (NB == 2 * NCORES)
                NJ = NBLK // B if PIPE else NBLK
                NB_A2A = B if PIPE else 1
                a2a_ins = [dram.tile([NCORES, NJ, P, GW], BF16,
                                     name=f"a2a_in{i}") for i in range(NB_A2A)]
                a2a_outs = [dram.tile([NCORES, NJ, P, GW], BF16,
                                      name=f"a2a_out{i}")
                            for i in range(NB_A2A)]
                att_tm = persist.tile([P, NBLK, D], BF16, tag="tm_buf")
                attT = persist.tile([P, DK, TOK], BF16, tag="fm_buf")

                def qgroups_of(b):
                    out = []
                    for c in range(NCORES):
                        mem = sorted(
                            (i, qb) for (bb, qb), (cc, i) in own.items()
                            if bb == b and cc == c)
                        qbs = [qb for _, qb in mem]
                        if not qbs:
                            continue
                        assert all(q1 < q2 for q1, q2 in zip(qbs, qbs[1:]))
                        col0 = tokcol(b, qbs[0])
                        for j, qb in enumerate(qbs):
                            assert tokcol(b, qb) == col0 + j * P
                        out.append((qbs, col0))
                    return out

                with (
                    tc.tile_pool(name="ps_s", bufs=2, space="PSUM") as ps_sp,
                    tc.tile_pool(name="ps_av", bufs=1, space="PSUM") as ps_avp,
                ):
                    for b in range(B):
                        for qbs, col0 in qgroups_of(b):
                            M = len(qbs)
                            pav = [[ps_avp.tile([P, 512], F32,
                                                tag=f"av{m}_{s}",
                                                name=f"pav{m}_{s}")[:, 0:HD + 1]
                                    for s in range(HPC)] for m in range(M)]
                            for kb in range(qbs[-1] + 1):
                                kcol = tokcol(b, kb)
                                fa = next(m for m in range(M) if qbs[m] >= kb)
                                qoff = col0 + fa * P
                                N = (M - fa) * P
                                ps2 = ps_sp.tile([P, HPC, 512], F32,
                                                 tag="score", name="ps2")
                                if attn_sub < 2:
                                    continue
                                for sub in range(HPC):
                                    nc.tensor.matmul(
                                        ps2[:, sub, 0:N],
                                        kT[sub * HD:(sub + 1) * HD,
                                           kcol:kcol + P],
                                        qT[sub * HD:(sub + 1) * HD,
                                           qoff:qoff + N],
                                        start=True, stop=True,
                                        tile_position=(sub * HD, 0))
                                p_sb = work.tile([P, HPC, 2 * P], BF16,
                                                 tag="p_sb", name="p_sb")
                                nc.scalar.activation(
                                    p_sb[:, :, 0:N], ps2[:, :, 0:N], AF.Exp,
                                    scale=float(HD) ** -0.5)
                                if attn_sub >= 3:
                                    for m in range(fa, M):
                                        if qbs[m] != kb:
                                            continue
                                        moff = (m - fa) * P
                                        for sub in range(HPC):
                                            nc.vector.tensor_tensor(
                                                p_sb[:, sub, moff:moff + P],
                                                p_sb[:, sub, moff:moff + P],
                                                tril[:], OP.mult)
                                slot = (b * NB + kb) * HPC
                                if attn_sub >= 4:
                                    for m in range(fa, M):
                                        moff = (m - fa) * P
                                        for sub in range(HPC):
                                            nc.tensor.matmul(
                                                pav[m][sub][:],
                                                p_sb[:, sub, moff:moff + P],
                                                v_aug[:, slot + sub, :],
                                                start=(kb == 0),
                                                stop=(kb == qbs[m]))
                            for m in range(M):
                                dst_c, dst_i = blkidx(b, qbs[m])
                                att = work.tile([P, GW], BF16, tag="att",
                                                name="att")
                                if attn_sub >= 5:
                                    for sub in range(HPC):
                                        rec = small.tile([P, 1], F32,
                                                         tag="rec",
                                                         name="rec")
                                        nc.vector.reciprocal(
                                            rec[:], pav[m][sub][:, HD:HD + 1])
                                        nc.vector.tensor_scalar_mul(
                                            att[:, sub * HD:(sub + 1) * HD],
                                            pav[m][sub][:, 0:HD],
                                            scalar1=rec[:])
                                else:
                                    nc.vector.memset(att[:], 0.5)
                                ai = b if PIPE else 0
                                aj = dst_i - b * NJ if PIPE else dst_i
                                nc.sync.dma_start(a2a_ins[ai][dst_c, aj],
                                                  att[:])

                        if active("a2a") and (PIPE or b == B - 1):
                            ai = b if PIPE else 0
                            nc.gpsimd.collective_compute(
                                "AllToAll", OP.bypass,
                                replica_groups=[list(range(NCORES))],
                                ins=[a2a_ins[ai].opt()],
                                outs=[a2a_outs[ai].opt()])
                            t0 = b * NJ if PIPE else 0
                            for r in range(NCORES):
                                for j in range(NJ):
                                    nc.gpsimd.dma_start(
                                        att_tm[:, t0 + j,
                                               r * GW:(r + 1) * GW],
                                        a2a_outs[ai][r, j])

            if active("a2a"):
                with tc.tile_pool(name="ps_tr2", bufs=4,
                                  space="PSUM") as ps_tr2:
                    for t in range(NBLK):
                        for d in range(DK):
                            pe_transpose(ps_tr2,
                                         attT[:, d, t * P:(t + 1) * P],
                                         att_tm[:, t, d * P:(d + 1) * P])

            if active("proj"):
                # ------------- stage D: proj + residual -> x2 -------------
                bproj_bc = persist.tile([P, D], F32)
                nc.scalar.dma_start(bproj_bc[:], bcast_rows(bproj))
                wproj_sb = persist.tile([P, DK, D], BF16)
                nc.scalar.dma_start(wproj_sb[:],
                                  wproj.rearrange("d p f -> p d f"))
                x2 = persist.tile([P, NBLK, D], F32)
                with tc.tile_pool(name="ps2", bufs=2, space="PSUM") as ps2:
                    for t in range(NBLK):
                        for n in range(D // 512):
                            ps = ps2.tile([P, 512], F32, tag="mm", name="ps")
                            for d in range(DK):
                                nc.tensor.matmul(
                                    ps[:], attT[:, d, t * P:(t + 1) * P],
                                    wproj_sb[:, d, n * 512:(n + 1) * 512],
                                    start=(d == 0), stop=(d == DK - 1))
                            sl = slice(n * 512, (n + 1) * 512)
                            nc.vector.tensor_tensor(
                                x2[:, t, sl], ps[:], x_res[:, t, sl], OP.add)
                            nc.vector.tensor_tensor(
                                x2[:, t, sl], x2[:, t, sl], bproj_bc[:, sl],
                                OP.add)

            if active("ln2"):
                # ---------------- stage E: LN2 -> h2T ----------------
                h2_tm = persist.tile([P, NBLK, D], BF16, tag="tm_buf")
                for t in range(NBLK):
                    st = small.tile([P, 2, 6], F32, tag="ln_st")
                    xg = x2[:, t, :].rearrange("p (s f) -> p s f", s=2)
                    for s in range(2):
                        nc.vector.bn_stats(out=st[:, s, :], in_=xg[:, s, :])
                    mv = small.tile([P, 2], F32, tag="ln_mv")
                    nc.vector.bn_aggr(out=mv[:], in_=st[:])
                    rstd = small.tile([P, 1], F32, tag="ln_rstd")
                    nc.scalar.activation(rstd[:], mv[:, 1:2], AF.Sqrt,
                                         bias=eps_t[:])
                    nc.vector.reciprocal(rstd[:], rstd[:])
                    nc.vector.tensor_scalar(
                        h2_tm[:, t, :], x2[:, t, :],
                        scalar1=mv[:, 0:1], scalar2=rstd[:],
                        op0=OP.subtract, op1=OP.mult)
                h2T = persist.tile([P, DK, TOK], BF16, tag="fm_buf")
                with tc.tile_pool(name="ps_tr3", bufs=4,
                                  space="PSUM") as ps_tr3:
                    for t in range(NBLK):
                        for d in range(DK):
                            pe_transpose(ps_tr3,
                                         h2T[:, d, t * P:(t + 1) * P],
                                         h2_tm[:, t, d * P:(d + 1) * P])

            if active("ff1"):
                # ---------------- stage F1: FFN up + gelu ----------------
                b1_sb = persist.tile([P, NFF], F32)
                nc.scalar.dma_start(b1_sb[:], b1e.rearrange("(c p) -> p c", p=P))
                gT = persist.tile([P, NFF, TOK], BF16, tag="big_buf")
                with tc.tile_pool(name="ps3", bufs=2, space="PSUM") as ps3:
                    for f in range(NFF):
                        ps = ps3.tile([P, TOK], F32, tag="mm", name="ps")
                        w1t = wstream.tile([P, DK, P], BF16, tag="w1_t",
                                           name="w1t")
                        nc.scalar.dma_start(
                            w1t[:], w1[:, :, f * P:(f + 1) * P]
                            .rearrange("d p f -> p d f"))
                        for d in range(DK):
                            nc.tensor.matmul(
                                ps[:], w1t[:, d, :], h2T[:, d, :],
                                start=(d == 0), stop=(d == DK - 1))
                        nc.scalar.activation(gT[:, f, :], ps[:], AF.Gelu,
                                             bias=b1_sb[:, f:f + 1])

            if active("ff2"):
                # ---------------- stage F2: FFN down + residual ------------
                b2_bc = persist.tile([P, D], F32)
                nc.scalar.dma_start(b2_bc[:], bcast_rows(b2))
                NOUT = D // 512
                with tc.tile_pool(name="ps4", bufs=1, space="PSUM") as ps4:
                    ps_out = [ps4.tile([P, 512], F32, tag=f"ff2_{i}",
                                       name=f"ff2_{i}")
                              for i in range(NBLK * NOUT)]
                    for f in range(NFF):
                        w2t = wstream.tile([P, D], BF16, tag="w2_t",
                                           name="w2t")
                        nc.scalar.dma_start(w2t[:], w2[f])
                        for t in range(NBLK):
                            for n in range(NOUT):
                                nc.tensor.matmul(
                                    ps_out[t * NOUT + n][:],
                                    gT[:, f, t * P:(t + 1) * P],
                                    w2t[:, n * 512:(n + 1) * 512],
                                    start=(f == 0), stop=(f == NFF - 1))
                    for t in range(NBLK):
                        for n in range(NOUT):
                            sl = slice(n * 512, (n + 1) * 512)
                            o = work.tile([P, 512], F32, tag="out_sb",
                                          name="o")
                            nc.vector.tensor_tensor(
                                o[:], ps_out[t * NOUT + n][:], x2[:, t, sl],
                                OP.add)
                            nc.vector.tensor_tensor(o[:], o[:], b2_bc[:, sl],
                                                    OP.add)
                            nc.sync.dma_start(out[t][:, sl], o[:])
            else:
                # truncated build (debug): write deterministic junk to out
                for t in range(NBLK):
                    o = work.tile([P, D], F32, tag="dbg_out", name="o")
                    nc.vector.memset(o[:], 1.0 + t)
                    nc.sync.dma_start(out[t], o[:])

    nc.compile()
    meta = dict(T=T, B=B, D=D, H=H, FF=FF, NB=NB, own=own, HPC=HPC,
                NBLK=NBLK, TOK=TOK, DK=DK, NFF=NFF)
    return nc, meta


_BUILT = {}


def _get_built(T, B, stop_after=None, attn_sub=5):
    key = (T, B, stop_after, attn_sub)
    if key not in _BUILT:
        _BUILT[key] = build(T=T, B=B, stop_after=stop_after, attn_sub=attn_sub)
    return _BUILT[key]


def _prep_inputs(inputs, meta):
    """Host-side: fold LN params into weights, cast bf16, shard per core."""
    B, T, D = meta["B"], meta["T"], meta["D"]
    H, FF = meta["H"], meta["FF"]
    HD = D // H
    NB, NBLK, own = meta["NB"], meta["NBLK"], meta["own"]
    DK, NFF, HPC = meta["DK"], meta["NFF"], meta["HPC"]

    f32 = np.float32
    bf = ml_dtypes.bfloat16
    x = np.asarray(inputs["x"], f32)
    g1 = np.asarray(inputs["ln1_g"], f32)
    b1n = np.asarray(inputs["ln1_b"], f32)
    g2 = np.asarray(inputs["ln2_g"], f32)
    b2n = np.asarray(inputs["ln2_b"], f32)
    wq = np.asarray(inputs["wq"], f32)   # [H, D, HD]
    wk = np.asarray(inputs["wk"], f32)
    wv = np.asarray(inputs["wv"], f32)

    wq_f = wq * g1[None, :, None]
    wk_f = wk * g1[None, :, None]
    wv_f = wv * g1[None, :, None]
    bq = np.einsum("c,hcd->hd", b1n, wq)
    bk = np.einsum("c,hcd->hd", b1n, wk)
    bv = np.einsum("c,hcd->hd", b1n, wv)
    assert np.abs(bv).max() == 0.0, "nonzero folded V bias unsupported"

    WQ = wq_f.transpose(1, 0, 2).reshape(D, D)   # [D, H*HD] head-major cols
    WK = wk_f.transpose(1, 0, 2).reshape(D, D)
    WV = wv_f.transpose(1, 0, 2).reshape(D, D)
    GW = HPC * HD
    wqkv_cores, qkvb_cores = [], []
    for c in range(NCORES):
        cs = slice(c * GW, (c + 1) * GW)
        wcat = np.concatenate([WQ[:, cs], WK[:, cs], WV[:, cs]], axis=1)
        wqkv_cores.append(
            np.ascontiguousarray(wcat.reshape(DK, P, 3 * GW)).astype(bf))
        qb_ = np.concatenate([bq.reshape(-1)[cs], bk.reshape(-1)[cs],
                              bv.reshape(-1)[cs]]).astype(f32)
        qkvb_cores.append(np.ascontiguousarray(qb_))

    w_proj = np.asarray(inputs["w_proj"], f32)
    b_proj = np.ascontiguousarray(np.asarray(inputs["b_proj"], f32))
    w1f = np.asarray(inputs["w1"], f32)
    w1 = w1f * g2[:, None]
    b1e = (np.asarray(inputs["b1"], f32) + b2n @ w1f).astype(f32)
    w2 = np.asarray(inputs["w2"], f32)
    b2 = np.ascontiguousarray(np.asarray(inputs["b2"], f32))

    wproj_d = np.ascontiguousarray(w_proj.reshape(DK, P, D)).astype(bf)
    w1_d = np.ascontiguousarray(w1.reshape(DK, P, FF)).astype(bf)
    w2_d = np.ascontiguousarray(w2.reshape(NFF, P, D)).astype(bf)
    b1e = np.ascontiguousarray(b1e)

    in_maps = []
    for c in range(NCORES):
        xc = np.empty((NBLK, P, D), f32)
        for (b, qb), (cc, i) in own.items():
            if cc != c:
                continue
            xc[i] = x[b, qb * P:(qb + 1) * P, :]
        in_maps.append({
            "xin": xc,
            "wqkv": wqkv_cores[c],
            "qkvb": qkvb_cores[c],
            "wproj": wproj_d,
            "bproj": b_proj,
            "w1": w1_d,
            "b1e": b1e,
            "w2": w2_d,
            "b2": b2,
        })
    return in_maps


def _gather_output(results, meta):
    B, T, D = meta["B"], meta["T"], meta["D"]
    NB, NBLK, own = meta["NB"], meta["NBLK"], meta["own"]
    out = np.empty((B, T, D), np.float32)
    for (b, qb), (c, i) in own.items():
        out[b, qb * P:(qb + 1) * P, :] = results[c]["out"][i]
    return out


def run(inputs, T=2048, B=2, trace=False, stop_after=None, attn_sub=5,
        **spmd_kwargs):
    nc, meta = _get_built(T, B, stop_after, attn_sub)
    in_maps = _prep_inputs(inputs, meta)
    res = run_bass_kernel_spmd(
        nc, in_maps, core_ids=list(range(NCORES)), trace=trace, **spmd_kwargs)
    return _gather_output(res.results, meta), res


def kernel(**inputs):
    out, _ = run(inputs, T=2048, B=2, trace=False)
    return out


# revision 21
# speedup vs baseline: 1.4383x; 1.0040x over previous
"""Trainium2 Bass kernel for a dense transformer block (pre-LN, causal MHA + FFN).

Reference shapes: x [B=2, T=2048, D=1024], H=16 heads, HD=64, FF=4096, f32.

Sharding (8 NeuronCores, SPMD single program):
  - Tokens (B*T = 32 blocks of 128) are owned by cores: core c owns blocks
    (c, 15-c) of each batch -> 512 tokens/core. LN1/LN2, residuals, proj and
    FFN are pure data-parallel over tokens.
  - Attention is head-sharded (2 heads/core over ALL tokens) so the causal
    loop structure is identical on every core. Two cheap collectives glue the
    shardings: AllGather of the LN1 output (feature-major, bf16, 1MB/rank)
    before QKV, and AllToAll of the attention output (1MB/rank) after.
  - Weights are cast to bf16 on host; LN gains/biases are folded into the
    adjacent weight matrices on host. Matmuls run in bf16 (fp32 accumulate),
    the residual stream stays f32 on device.
"""
import numpy as np
import ml_dtypes

import concourse.bass as bass
from concourse import bacc
import concourse.tile as tile
import concourse.mybir as mybir
from concourse.bass_utils import run_bass_kernel_spmd
from concourse.masks import make_identity

F32 = mybir.dt.float32
BF16 = mybir.dt.bfloat16
AF = mybir.ActivationFunctionType
OP = mybir.AluOpType

P = 128
NCORES = 8

STAGES = ["ln1", "ag", "qkv", "attn", "a2a", "proj", "ln2", "ff1", "ff2"]


def _owner_map(b_total, nb):
    """(batch, block) -> (core, local block idx). Full size pairs block i with
    nb-1-i on one core so causal attention work is balanced."""
    own = {}
    if nb == 2 * NCORES:
        for b in range(b_total):
            for qb in range(nb):
                c = qb if qb < NCORES else nb - 1 - qb
                own[(b, qb)] = (c, 2 * b + (0 if qb < NCORES else 1))
    else:
        for b in range(b_total):
            for qb in range(nb):
                inst = b * nb + qb
                own[(b, qb)] = (inst % NCORES, inst // NCORES)
    return own


def build(T=2048, B=2, D=1024, H=16, FF=4096, eps=1e-5, stop_after=None, attn_sub=5):
    HD = D // H
    NB = T // P                    # blocks per batch
    HPC = H // NCORES              # heads per core
    NBLK = B * NB // NCORES        # owned token blocks per core
    TOK = NBLK * P                 # owned tokens per core
    TQ = NCORES * TOK              # all tokens (B*T)
    DK = D // P                    # d_model chunks
    NFF = FF // P                  # ff chunks
    NKB = B * NB                   # total key blocks
    own = _owner_map(B, NB)
    GW = HPC * HD                  # my attention feature width (128)
    assert GW == P

    if stop_after is None:
        stop_after = "ff2"
    assert stop_after in STAGES

    def active(stage):
        return STAGES.index(stage) <= STAGES.index(stop_after)

    def blkidx(b, qb):
        return own[(b, qb)]

    def tokcol(b, qb):
        c, i = blkidx(b, qb)
        return c * TOK + i * P

    nc = bacc.Bacc("TRN2", num_devices=NCORES)

    # ---- DRAM I/O (per core) ----
    xin = nc.dram_tensor("xin", [NBLK, P, D], F32, kind="ExternalInput").ap()
    wqkv = nc.dram_tensor("wqkv", [DK, P, 3 * GW], BF16, kind="ExternalInput").ap()
    qkvb = nc.dram_tensor("qkvb", [3 * GW], F32, kind="ExternalInput").ap()
    wproj = nc.dram_tensor("wproj", [DK, P, D], BF16, kind="ExternalInput").ap()
    bproj = nc.dram_tensor("bproj", [D], F32, kind="ExternalInput").ap()
    w1 = nc.dram_tensor("w1", [DK, P, FF], BF16, kind="ExternalInput").ap()
    b1e = nc.dram_tensor("b1e", [FF], F32, kind="ExternalInput").ap()
    w2 = nc.dram_tensor("w2", [NFF, P, D], BF16, kind="ExternalInput").ap()
    b2 = nc.dram_tensor("b2", [D], F32, kind="ExternalInput").ap()
    out = nc.dram_tensor("out", [NBLK, P, D], F32, kind="ExternalOutput").ap()

    def bcast_rows(vec_ap):
        return bass.AP(tensor=vec_ap.tensor, offset=vec_ap.offset,
                       ap=[[0, P]] + list(vec_ap.ap))

    with tile.TileContext(nc) as tc:
        with (
            tc.tile_pool(name="persist", bufs=1) as persist,
            tc.tile_pool(name="dram", bufs=1, space="DRAM") as dram,
            tc.tile_pool(name="wstream", bufs=3) as wstream,
            tc.tile_pool(name="work", bufs=5) as work,
            tc.tile_pool(name="small", bufs=6) as small,
        ):
            ident = persist.tile([P, P], BF16)
            make_identity(nc, ident[:])
            tril = persist.tile([P, P], BF16)
            nc.vector.memset(tril[:], 1.0)
            nc.gpsimd.affine_select(
                out=tril[:], in_=tril[:], compare_op=OP.is_ge,
                fill=0.0, base=0, channel_multiplier=-1, pattern=[[1, P]])

            def pe_transpose(pool, dst_slice, src_slice):
                pt = pool.tile([P, P], BF16, tag="tr", name="pt")
                nc.tensor.transpose(pt[:], src_slice, ident[:])
                nc.vector.tensor_copy(dst_slice, pt[:])

            # ---------------- stage A: LN1 over own tokens ----------------
            x_res = persist.tile([P, NBLK, D], F32)
            h_tm = persist.tile([P, NBLK, D], BF16, tag="tm_buf")
            eps_t = persist.tile([P, 1], F32)
            nc.vector.memset(eps_t[:], eps)
            for t in range(NBLK):
                nc.sync.dma_start(x_res[:, t, :], xin[t])
                st = small.tile([P, 2, 6], F32, tag="ln_st")
                xg = x_res[:, t, :].rearrange("p (s f) -> p s f", s=2)
                for s in range(2):
                    nc.vector.bn_stats(out=st[:, s, :], in_=xg[:, s, :])
                mv = small.tile([P, 2], F32, tag="ln_mv")
                nc.vector.bn_aggr(out=mv[:], in_=st[:])
                rstd = small.tile([P, 1], F32, tag="ln_rstd")
                nc.scalar.activation(rstd[:], mv[:, 1:2], AF.Sqrt, bias=eps_t[:])
                nc.vector.reciprocal(rstd[:], rstd[:])
                nc.vector.tensor_scalar(
                    h_tm[:, t, :], x_res[:, t, :],
                    scalar1=mv[:, 0:1], scalar2=rstd[:],
                    op0=OP.subtract, op1=OP.mult)

            # transpose h -> feature-major [P, DK, TOK] via PE transpose
            hT_own = persist.tile([P, DK, TOK], BF16, tag="fm_buf")
            with tc.tile_pool(name="ps_tr1", bufs=4, space="PSUM") as ps_tr1:
                for d in range(DK):
                    for t in range(NBLK):
                        pe_transpose(ps_tr1,
                                     hT_own[:, d, t * P:(t + 1) * P],
                                     h_tm[:, t, d * P:(d + 1) * P])

            if active("ag"):
                # ---------------- AllGather hT ----------------
                ag_in = dram.tile([DK, P, TOK], BF16)
                ag_out = dram.tile([NCORES, DK, P, TOK], BF16,
                                   addr_space="Shared")
                for d in range(DK):
                    nc.sync.dma_start(ag_in[d], hT_own[:, d, :])
                nc.gpsimd.collective_compute(
                    "AllGather", OP.bypass,
                    replica_groups=[list(range(NCORES))],
                    ins=[ag_in.opt()], outs=[ag_out.opt()])
                hT = persist.tile([P, DK, TQ], BF16, tag="big_buf")
                for r in range(NCORES):
                    for d in range(DK):
                        nc.sync.dma_start(
                            hT[:, d, r * TOK:(r + 1) * TOK], ag_out[r, d])

            if active("qkv"):
                # ---------------- stage B: QKV for my heads ----------------
                qT = persist.tile([P, TQ], BF16)
                kT = persist.tile([P, TQ], BF16)
                qkvb_sb = persist.tile([P, 3], F32)
                nc.scalar.dma_start(qkvb_sb[:],
                                  qkvb.rearrange("(g p) -> p g", p=P))
                wqkv_sb = persist.tile([P, DK, 3 * GW], BF16)
                nc.scalar.dma_start(wqkv_sb[:], wqkv.rearrange("d p f -> p d f"))
                NT = TQ // 512
                with tc.tile_pool(name="ps1", bufs=2, space="PSUM") as ps1:
                    for g, dst in ((0, qT), (1, kT)):
                        for nt in range(NT):
                            ps = ps1.tile([P, 512], F32, tag="mm", name="ps")
                            for d in range(DK):
                                nc.tensor.matmul(
                                    ps[:], wqkv_sb[:, d, g * P:(g + 1) * P],
                                    hT[:, d, nt * 512:(nt + 1) * 512],
                                    start=(d == 0), stop=(d == DK - 1))
                            nc.vector.tensor_scalar_add(
                                dst[:, nt * 512:(nt + 1) * 512], ps[:],
                                scalar1=qkvb_sb[:, g:g + 1])

                    # V token-major with fused ones column
                    # (folded LN1 bias for V must be zero; host asserts)
                    v_aug = persist.tile([P, NKB * HPC, HD + 1], BF16)
                    nc.vector.memset(v_aug[:, :, HD:HD + 1], 1.0)
                    for b in range(B):
                        for kb in range(NB):
                            col = tokcol(b, kb)
                            slot = (b * NB + kb) * HPC
                            ps = ps1.tile([P, 512], F32, tag="mm", name="ps")
                            for d in range(DK):
                                nc.tensor.matmul(
                                    ps[:, 0:GW], hT[:, d, col:col + P],
                                    wqkv_sb[:, d, 2 * P:3 * P],
                                    start=(d == 0), stop=(d == DK - 1))
                            for sub in range(HPC):
                                nc.vector.tensor_copy(
                                    v_aug[:, slot + sub, 0:HD],
                                    ps[:, sub * HD:(sub + 1) * HD])

            if active("attn"):
                # ------------- stage C: attention (my HPC heads) -----------
                # Per-batch AllToAll at full size so batch-0's A2A + attT
                # assembly overlap batch-1's attention compute.
                PIPE = (NB == 2 * NCORES)
                NJ = NBLK // B if PIPE else NBLK
                NB_A2A = B if PIPE else 1
                a2a_ins = [dram.tile([NCORES, NJ, P, GW], BF16,
                                     name=f"a2a_in{i}") for i in range(NB_A2A)]
                a2a_outs = [dram.tile([NCORES, NJ, P, GW], BF16,
                                      name=f"a2a_out{i}")
                            for i in range(NB_A2A)]
                att_tm = persist.tile([P, NBLK, D], BF16, tag="tm_buf")
                attT = persist.tile([P, DK, TOK], BF16, tag="fm_buf")

                def qgroups_of(b):
                    out = []
                    for c in range(NCORES):
                        mem = sorted(
                            (i, qb) for (bb, qb), (cc, i) in own.items()
                            if bb == b and cc == c)
                        qbs = [qb for _, qb in mem]
                        if not qbs:
                            continue
                        assert all(q1 < q2 for q1, q2 in zip(qbs, qbs[1:]))
                        col0 = tokcol(b, qbs[0])
                        for j, qb in enumerate(qbs):
                            assert tokcol(b, qb) == col0 + j * P
                        out.append((qbs, col0))
                    return out

                with (
                    tc.tile_pool(name="ps_s", bufs=2, space="PSUM") as ps_sp,
                    tc.tile_pool(name="ps_av", bufs=1, space="PSUM") as ps_avp,
                ):
                    for b in range(B):
                        for qbs, col0 in qgroups_of(b):
                            M = len(qbs)
                            pav = [[ps_avp.tile([P, 512], F32,
                                                tag=f"av{m}_{s}",
                                                name=f"pav{m}_{s}")[:, 0:HD + 1]
                                    for s in range(HPC)] for m in range(M)]
                            for kb in range(qbs[-1] + 1):
                                kcol = tokcol(b, kb)
                                fa = next(m for m in range(M) if qbs[m] >= kb)
                                qoff = col0 + fa * P
                                N = (M - fa) * P
                                ps2 = ps_sp.tile([P, HPC, 512], F32,
                                                 tag="score", name="ps2")
                                if attn_sub < 2:
                                    continue
                                for sub in range(HPC):
                                    nc.tensor.matmul(
                                        ps2[:, sub, 0:N],
                                        kT[sub * HD:(sub + 1) * HD,
                                           kcol:kcol + P],
                                        qT[sub * HD:(sub + 1) * HD,
                                           qoff:qoff + N],
                                        start=True, stop=True,
                                        tile_position=(sub * HD, 0))
                                p_sb = work.tile([P, HPC, 2 * P], BF16,
                                                 tag="p_sb", name="p_sb")
                                nc.scalar.activation(
                                    p_sb[:, :, 0:N], ps2[:, :, 0:N], AF.Exp,
                                    scale=float(HD) ** -0.5)
                                if attn_sub >= 3:
                                    for m in range(fa, M):
                                        if qbs[m] != kb:
                                            continue
                                        moff = (m - fa) * P
                                        for sub in range(HPC):
                                            nc.vector.tensor_tensor(
                                                p_sb[:, sub, moff:moff + P],
                                                p_sb[:, sub, moff:moff + P],
                                                tril[:], OP.mult)
                                slot = (b * NB + kb) * HPC
                                if attn_sub >= 4:
                                    for m in range(fa, M):
                                        moff = (m - fa) * P
                                        for sub in range(HPC):
                                            nc.tensor.matmul(
                                                pav[m][sub][:],
                                                p_sb[:, sub, moff:moff + P],
                                                v_aug[:, slot + sub, :],
                                                start=(kb == 0),
                                                stop=(kb == qbs[m]))
                            for m in range(M):
                                dst_c, dst_i = blkidx(b, qbs[m])
                                att = work.tile([P, GW], BF16, tag="att",
                                                name="att")
                                if attn_sub >= 5:
                                    for sub in range(HPC):
                                        rec = small.tile([P, 1], F32,
                                                         tag="rec",
                                                         name="rec")
                                        nc.vector.reciprocal(
                                            rec[:], pav[m][sub][:, HD:HD + 1])
                                        nc.vector.tensor_scalar_mul(
                                            att[:, sub * HD:(sub + 1) * HD],
                                            pav[m][sub][:, 0:HD],
                                            scalar1=rec[:])
                                else:
                                    nc.vector.memset(att[:], 0.5)
                                ai = b if PIPE else 0
                                aj = dst_i - b * NJ if PIPE else dst_i
                                nc.sync.dma_start(a2a_ins[ai][dst_c, aj],
                                                  att[:])

                        if active("a2a") and (PIPE or b == B - 1):
                            ai = b if PIPE else 0
                            nc.gpsimd.collective_compute(
                                "AllToAll", OP.bypass,
                                replica_groups=[list(range(NCORES))],
                                ins=[a2a_ins[ai].opt()],
                                outs=[a2a_outs[ai].opt()])
                            t0 = b * NJ if PIPE else 0
                            for r in range(NCORES):
                                for j in range(NJ):
                                    nc.gpsimd.dma_start(
                                        att_tm[:, t0 + j,
                                               r * GW:(r + 1) * GW],
                                        a2a_outs[ai][r, j])

            if active("a2a"):
                with tc.tile_pool(name="ps_tr2", bufs=4,
                                  space="PSUM") as ps_tr2:
                    for t in range(NBLK):
                        for d in range(DK):
                            pe_transpose(ps_tr2,
                                         attT[:, d, t * P:(t + 1) * P],
                                         att_tm[:, t, d * P:(d + 1) * P])

            if active("proj"):
                # ------------- stage D: proj + residual -> x2 -------------
                bproj_bc = persist.tile([P, D], F32)
                nc.scalar.dma_start(bproj_bc[:], bcast_rows(bproj))
                wproj_sb = persist.tile([P, DK, D], BF16)
                nc.scalar.dma_start(wproj_sb[:],
                                  wproj.rearrange("d p f -> p d f"))
                x2 = persist.tile([P, NBLK, D], F32)
                with tc.tile_pool(name="ps2", bufs=2, space="PSUM") as ps2:
                    for t in range(NBLK):
                        for n in range(D // 512):
                            ps = ps2.tile([P, 512], F32, tag="mm", name="ps")
                            for d in range(DK):
                                nc.tensor.matmul(
                                    ps[:], attT[:, d, t * P:(t + 1) * P],
                                    wproj_sb[:, d, n * 512:(n + 1) * 512],
                                    start=(d == 0), stop=(d == DK - 1))
                            sl = slice(n * 512, (n + 1) * 512)
                            nc.vector.tensor_tensor(
                                x2[:, t, sl], ps[:], x_res[:, t, sl], OP.add)
                            nc.vector.tensor_tensor(
                                x2[:, t, sl], x2[:, t, sl], bproj_bc[:, sl],
                                OP.add)

            if active("ln2"):
                # ---------------- stage E: LN2 -> h2T ----------------
                h2_tm = persist.tile([P, NBLK, D], BF16, tag="tm_buf")
                for t in range(NBLK):
                    st = small.tile([P, 2, 6], F32, tag="ln_st")
                    xg = x2[:, t, :].rearrange("p (s f) -> p s f", s=2)
                    for s in range(2):
                        nc.vector.bn_stats(out=st[:, s, :], in_=xg[:, s, :])
                    mv = small.tile([P, 2], F32, tag="ln_mv")
                    nc.vector.bn_aggr(out=mv[:], in_=st[:])
                    rstd = small.tile([P, 1], F32, tag="ln_rstd")
                    nc.scalar.activation(rstd[:], mv[:, 1:2], AF.Sqrt,
                                         bias=eps_t[:])
                    nc.vector.reciprocal(rstd[:], rstd[:])
                    nc.vector.tensor_scalar(
                        h2_tm[:, t, :], x2[:, t, :],
                        scalar1=mv[:, 0:1], scalar2=rstd[:],
                        op0=OP.subtract, op1=OP.mult)
                h2T = persist.tile([P, DK, TOK], BF16, tag="fm_buf")
                with tc.tile_pool(name="ps_tr3", bufs=4,
                                  space="PSUM") as ps_tr3:
                    for t in range(NBLK):
                        for d in range(DK):
                            pe_transpose(ps_tr3,
                                         h2T[:, d, t * P:(t + 1) * P],
                                         h2_tm[:, t, d * P:(d + 1) * P])

            if active("ff1"):
                # ---------------- stage F1: FFN up + gelu ----------------
                b1_sb = persist.tile([P, NFF], F32)
                nc.scalar.dma_start(b1_sb[:], b1e.rearrange("(c p) -> p c", p=P))
                gT = persist.tile([P, NFF, TOK], BF16, tag="big_buf")
                with tc.tile_pool(name="ps3", bufs=2, space="PSUM") as ps3:
                    for f in range(NFF):
                        ps = ps3.tile([P, TOK], F32, tag="mm", name="ps")
                        w1t = wstream.tile([P, DK, P], BF16, tag="w1_t",
                                           name="w1t")
                        nc.scalar.dma_start(
                            w1t[:], w1[:, :, f * P:(f + 1) * P]
                            .rearrange("d p f -> p d f"))
                        for d in range(DK):
                            nc.tensor.matmul(
                                ps[:], w1t[:, d, :], h2T[:, d, :],
                                start=(d == 0), stop=(d == DK - 1))
                        nc.scalar.activation(gT[:, f, :], ps[:], AF.Gelu,
                                             bias=b1_sb[:, f:f + 1])

            if active("ff2"):
                # ---------------- stage F2: FFN down + residual ------------
                b2_bc = persist.tile([P, D], F32)
                nc.scalar.dma_start(b2_bc[:], bcast_rows(b2))
                NOUT = D // 512
                with tc.tile_pool(name="ps4", bufs=1, space="PSUM") as ps4:
                    ps_out = [ps4.tile([P, 512], F32, tag=f"ff2_{i}",
                                       name=f"ff2_{i}")
                              for i in range(NBLK * NOUT)]
                    for f in range(NFF):
                        w2t = wstream.tile([P, D], BF16, tag="w2_t",
                                           name="w2t")
                        nc.scalar.dma_start(w2t[:], w2[f])
                        for t in range(NBLK):
                            for n in range(NOUT):
                                nc.tensor.matmul(
                                    ps_out[t * NOUT + n][:],
                                    gT[:, f, t * P:(t + 1) * P],
                                    w2t[:, n * 512:(n + 1) * 512],
                                    start=(f == 0), stop=(f == NFF - 1))
                    for t in range(NBLK):
                        for n in range(NOUT):
                            sl = slice(n * 512, (n + 1) * 512)
                            o = work.tile([P, 512], F32, tag="out_sb",
                                          name="o")
                            nc.vector.tensor_tensor(
                                o[:], ps_out[t * NOUT + n][:], x2[:, t, sl],
                                OP.add)
                            nc.vector.tensor_tensor(o[:], o[:], b2_bc[:, sl],
                                                    OP.add)
                            nc.sync.dma_start(out[t][:, sl], o[:])
            else:
                # truncated build (debug): write deterministic junk to out
                for t in range(NBLK):
                    o = work.tile([P, D], F32, tag="dbg_out", name="o")
                    nc.vector.memset(o[:], 1.0 + t)
                    nc.sync.dma_start(out[t], o[:])

    nc.compile()
    meta = dict(T=T, B=B, D=D, H=H, FF=FF, NB=NB, own=own, HPC=HPC,
                NBLK=NBLK, TOK=TOK, DK=DK, NFF=NFF)
    return nc, meta


_BUILT = {}


def _get_built(T, B, stop_after=None, attn_sub=5):
    key = (T, B, stop_after, attn_sub)
    if key not in _BUILT:
        _BUILT[key] = build(T=T, B=B, stop_after=stop_after, attn_sub=attn_sub)
    return _BUILT[key]


def _prep_inputs(inputs, meta):
    """Host-side: fold LN params into weights, cast bf16, shard per core."""
    B, T, D = meta["B"], meta["T"], meta["D"]
    H, FF = meta["H"], meta["FF"]
    HD = D // H
    NB, NBLK, own = meta["NB"], meta["NBLK"], meta["own"]
    DK, NFF, HPC = meta["DK"], meta["NFF"], meta["HPC"]

    f32 = np.float32
    bf = ml_dtypes.bfloat16
    x = np.asarray(inputs["x"], f32)
    g1 = np.asarray(inputs["ln1_g"], f32)
    b1n = np.asarray(inputs["ln1_b"], f32)
    g2 = np.asarray(inputs["ln2_g"], f32)
    b2n = np.asarray(inputs["ln2_b"], f32)
    wq = np.asarray(inputs["wq"], f32)   # [H, D, HD]
    wk = np.asarray(inputs["wk"], f32)
    wv = np.asarray(inputs["wv"], f32)

    wq_f = wq * g1[None, :, None]
    wk_f = wk * g1[None, :, None]
    wv_f = wv * g1[None, :, None]
    bq = np.einsum("c,hcd->hd", b1n, wq)
    bk = np.einsum("c,hcd->hd", b1n, wk)
    bv = np.einsum("c,hcd->hd", b1n, wv)
    assert np.abs(bv).max() == 0.0, "nonzero folded V bias unsupported"

    WQ = wq_f.transpose(1, 0, 2).reshape(D, D)   # [D, H*HD] head-major cols
    WK = wk_f.transpose(1, 0, 2).reshape(D, D)
    WV = wv_f.transpose(1, 0, 2).reshape(D, D)
    GW = HPC * HD
    wqkv_cores, qkvb_cores = [], []
    for c in range(NCORES):
        cs = slice(c * GW, (c + 1) * GW)
        wcat = np.concatenate([WQ[:, cs], WK[:, cs], WV[:, cs]], axis=1)
        wqkv_cores.append(
            np.ascontiguousarray(wcat.reshape(DK, P, 3 * GW)).astype(bf))
        qb_ = np.concatenate([bq.reshape(-1)[cs], bk.reshape(-1)[cs],
                              bv.reshape(-1)[cs]]).astype(f32)
        qkvb_cores.append(np.ascontiguousarray(qb_))

    w_proj = np.asarray(inputs["w_proj"], f32)
    b_proj = np.ascontiguousarray(np.asarray(inputs["b_proj"], f32))
    w1f = np.asarray(inputs["w1"], f32)
    w1 = w1f * g2[:, None]
    b1e = (np.asarray(inputs["b1"], f32) + b2n @ w1f).astype(f32)
    w2 = np.asarray(inputs["w2"], f32)
    b2 = np.ascontiguousarray(np.asarray(inputs["b2"], f32))

    wproj_d = np.ascontiguousarray(w_proj.reshape(DK, P, D)).astype(bf)
    w1_d = np.ascontiguousarray(w1.reshape(DK, P, FF)).astype(bf)
    w2_d = np.ascontiguousarray(w2.reshape(NFF, P, D)).astype(bf)
    b1e = np.ascontiguousarray(b1e)

    in_maps = []
    for c in range(NCORES):
        xc = np.empty((NBLK, P, D), f32)
        for (b, qb), (cc, i) in own.items():
            if cc != c:
                continue
            xc[i] = x[b, qb * P:(qb + 1) * P, :]
        in_maps.append({
            "xin": xc,
            "wqkv": wqkv_cores[c],
            "qkvb": qkvb_cores[c],
            "wproj": wproj_d,
            "bproj": b_proj,
            "w1": w1_d,
            "b1e": b1e,
            "w2": w2_d,
            "b2": b2,
        })
    return in_maps


def _gather_output(results, meta):
    B, T, D = meta["B"], meta["T"], meta["D"]
    NB, NBLK, own = meta["NB"], meta["NBLK"], meta["own"]
    out = np.empty((B, T, D), np.float32)
    for (b, qb), (c, i) in own.items():
        out[b, qb * P:(qb + 1) * P, :] = results[c]["out"][i]
    return out


def run(inputs, T=2048, B=2, trace=False, stop_after=None, attn_sub=5,
        **spmd_kwargs):
    nc, meta = _get_built(T, B, stop_after, attn_sub)
    in_maps = _prep_inputs(inputs, meta)
    res = run_bass_kernel_spmd(
        nc, in_maps, core_ids=list(range(NCORES)), trace=trace, **spmd_kwargs)
    return _gather_output(res.results, meta), res


def kernel(**inputs):
    out, _ = run(inputs, T=2048, B=2, trace=False)
    return out
